# revision 4
# baseline (speedup 1.0000x reference)
"""v15: v14 + split-d leaf layout with DMA-CCE fold, Pool mask-mult.

Host (data layout only, no FLOPs):
  Sort samples by context; 32 equal shards of 2048 (16 tiles of 128), core c
  runs shards 4c..4c+3.  For levels 0..14 the 128 sorted samples of a tile
  share few distinct ancestors, so each tile gets a node table (116 slot
  columns, per-level segment widths SLOTS) plus a per-sample one-hot mask.
  Leaf levels 15..20 rows and the z rows are pre-gathered per sample.

Device per shard (2048 samples):
  TensorE: psum[s, 116] = zt_tile^T @ table_tile   (dots vs all slot rows)
  ACT:     evacuate psum -> bf16
  Pool:    mm = ev * mask
  DVE:     l0-8 pair-sum; l9-14 segmented tensor_reduce -> logits 0..14
           prod[h, :] = rows * z (d split hi/lo); h-fold via DMA-CCE add
           (shard 0) or DVE add; halving tree -> logits 15..20
  ACT:     sigmoid ; DVE: product tree -> probs
"""

import sys

for _p in ("/opt/trn_rl_repo", "/root/.axon_site/_ro/trn_rl_repo"):
    if _p not in sys.path:
        sys.path.append(_p)

import ml_dtypes
import numpy as np

import concourse.bacc as bacc
import concourse.mybir as mybir
import concourse.tile as tile
from concourse.bass_utils import run_bass_kernel_spmd

N_CORES = 8
BATCH = 65536
DEPTH = 20
OFFSET = (1 << DEPTH) - 1
D = 128
P = 128

SPC = 4                     # shards per core
NSHARD = N_CORES * SPC      # 32
SH = BATCH // NSHARD        # 2048 samples per shard
TPS = SH // P               # 16 tiles per shard
NLEV_TAB = 15               # levels 0..14 via per-tile tables
NLEV_LEAF = 6               # levels 15..20 pre-gathered rows
NGRP = 4                    # psum groups per shard (4 tiles each)
TPG = TPS // NGRP
HD = D // 2                 # 64: leaf d-split half
FOLD_SHARDS = 0             # shards whose h-fold runs on DMA-CCE

SLOTS = [2] * 9 + [4, 6, 8, 12, 24, 44]
SEG_OFF = np.concatenate(([0], np.cumsum(SLOTS))).astype(np.int64)
NCOL = int(SEG_OFF[-1])     # 116

f32 = mybir.dt.float32
bf16 = mybir.dt.bfloat16
bfnp = ml_dtypes.bfloat16


def build_kernel():
    nc = bacc.Bacc("TRN2", target_bir_lowering=False, debug=False,
                   num_devices=N_CORES)

    ins = []
    for k in range(SPC):
        ins.append({
            "zp": nc.dram_tensor(f"zp_{k}", [P, TPS * D], bf16,
                                 kind="ExternalInput"),
            "zt": nc.dram_tensor(f"zt_{k}", [P, TPS * P], bf16,
                                 kind="ExternalInput"),
            "rl": nc.dram_tensor(f"rl_{k}", [P, TPS * NLEV_LEAF * D], bf16,
                                 kind="ExternalInput"),
            "mk": nc.dram_tensor(f"mk_{k}", [P, TPS * NCOL], bf16,
                                 kind="ExternalInput"),
            "tb": nc.dram_tensor(f"tb_{k}", [P, TPS * NCOL], bf16,
                                 kind="ExternalInput"),
        })
    out = nc.dram_tensor("out", [P, SPC * TPS], f32, kind="ExternalOutput")

    with tile.TileContext(nc) as tc:
        with (
            tc.tile_pool(name="const", bufs=1) as cpool,
            tc.tile_pool(name="stream", bufs=2) as spool,
            tc.tile_pool(name="evp", bufs=2) as epool,
            tc.tile_pool(name="mmp", bufs=2) as wpool,
            tc.tile_pool(name="prodp", bufs=2) as ppool,
            tc.tile_pool(name="halfp", bufs=1) as hpool,
            tc.tile_pool(name="logp", bufs=2) as lpool,
            tc.tile_pool(name="psum", bufs=4, space="PSUM") as qpool,
        ):
            probs = cpool.tile([P, SPC * TPS], f32)

            for k in range(SPC):
                t_in = ins[k]
                zt = spool.tile([P, TPS, P], bf16, tag="zt")
                nc.sync.dma_start(out=zt[:], in_=t_in["zt"].ap().rearrange(
                    "p (t s) -> p t s", s=P))
                tb = spool.tile([P, TPS, NCOL], bf16, tag="tb")
                nc.sync.dma_start(out=tb[:], in_=t_in["tb"].ap().rearrange(
                    "p (t c) -> p t c", c=NCOL))
                mk = spool.tile([P, TPS, NCOL], bf16, tag="mk")
                nc.sync.dma_start(out=mk[:], in_=t_in["mk"].ap().rearrange(
                    "p (t c) -> p t c", c=NCOL))
                zp = spool.tile([P, TPS, D], bf16, tag="zp")
                nc.sync.dma_start(out=zp[:], in_=t_in["zp"].ap().rearrange(
                    "p (t d) -> p t d", d=D))
                rl = spool.tile([P, TPS, NLEV_LEAF * D], bf16, tag="rl")
                nc.sync.dma_start(out=rl[:], in_=t_in["rl"].ap().rearrange(
                    "p (t x) -> p t x", x=NLEV_LEAF * D))

                lg = lpool.tile([P, TPS, 21], f32, tag="lg")

                # --- tabled levels 0..14: matmul -> psum -> bf16 evac ---
                ev = epool.tile([P, TPS, NCOL], bf16, tag="ev")
                for g in range(NGRP):
                    pt = qpool.tile([P, TPG, NCOL], f32, tag="pt")
                    for i in range(TPG):
                        t = g * TPG + i
                        nc.tensor.matmul(pt[:, i, :], zt[:, t, :], tb[:, t, :],
                                         start=True, stop=True)
                    nc.scalar.copy(out=ev[:, g * TPG:(g + 1) * TPG, :],
                                   in_=pt[:])

                mm = wpool.tile([P, TPS, NCOL], bf16, tag="mm")
                nc.gpsimd.tensor_tensor(out=mm[:], in0=ev[:], in1=mk[:],
                                        op=mybir.AluOpType.mult)
                # levels 0..8: nine 2-wide segments, one pair-add
                m2 = mm[:, :, 0:18].rearrange("p t (l r) -> p t l r", r=2)
                nc.vector.tensor_tensor(
                    out=lg[:, :, 0:9], in0=m2[:, :, :, 0], in1=m2[:, :, :, 1],
                    op=mybir.AluOpType.add)
                for lev in range(9, NLEV_TAB):
                    off = int(SEG_OFF[lev])
                    w = SLOTS[lev]
                    nc.vector.tensor_reduce(
                        out=lg[:, :, lev],
                        in_=mm[:, :, off:off + w],
                        axis=mybir.AxisListType.X, op=mybir.AluOpType.add)

                # --- leaf levels 15..20: d-split mult, h-fold, tree ---
                prod = ppool.tile([P, 2, TPS, NLEV_LEAF, HD], bf16,
                                  tag="prod")
                rlv = rl[:].rearrange("p t (l h d) -> p t l h d", h=2, d=HD)
                zpv = zp[:].rearrange("p t (h d) -> p t h d", h=2)
                for h in range(2):
                    nc.vector.tensor_tensor(
                        out=prod[:, h],
                        in0=zpv[:, :, h, :].unsqueeze(2).to_broadcast(
                            [P, TPS, NLEV_LEAF, HD]),
                        in1=rlv[:, :, :, h, :],
                        op=mybir.AluOpType.mult)
                if k < FOLD_SHARDS:
                    nc.gpsimd.dma_start(out=prod[:, 0], in_=prod[:, 1],
                                        accum_op=mybir.AluOpType.add)
                    cur = prod[:, 0]
                else:
                    ph = hpool.tile([P, TPS, NLEV_LEAF, HD], bf16, tag="ph64")
                    nc.vector.tensor_tensor(
                        out=ph[:], in0=prod[:, 0], in1=prod[:, 1],
                        op=mybir.AluOpType.add)
                    cur = ph[:]
                width = HD
                while width > 2:
                    width //= 2
                    nxt = hpool.tile([P, TPS, NLEV_LEAF, width], bf16,
                                     tag=f"ph{width}", name=f"ph{width}")
                    nc.vector.tensor_tensor(
                        out=nxt[:], in0=cur[:, :, :, 0:width],
                        in1=cur[:, :, :, width:2 * width],
                        op=mybir.AluOpType.add)
                    cur = nxt[:]
                nc.vector.tensor_tensor(
                    out=lg[:, :, NLEV_TAB:21],
                    in0=cur[:, :, :, 0], in1=cur[:, :, :, 1],
                    op=mybir.AluOpType.add)

                # --- sigmoid + product ---
                sg = lpool.tile([P, TPS, 32], f32, tag="sg")
                nc.vector.memset(sg[:, :, 21:32], 1.0)
                nc.scalar.activation(
                    out=sg[:, :, 0:21], in_=lg[:],
                    func=mybir.ActivationFunctionType.Sigmoid)
                cur, width = sg[:], 32
                while width > 2:
                    width //= 2
                    nxt = lpool.tile([P, TPS, width], f32, tag=f"s{width}",
                                     name=f"s{width}")
                    nc.vector.tensor_tensor(
                        out=nxt[:], in0=cur[:, :, 0:width],
                        in1=cur[:, :, width:2 * width],
                        op=mybir.AluOpType.mult)
                    cur = nxt[:]
                nc.vector.tensor_tensor(
                    out=probs[:, k * TPS:(k + 1) * TPS], in0=cur[:, :, 0],
                    in1=cur[:, :, 1], op=mybir.AluOpType.mult)
                nc.sync.dma_start(
                    out=out.ap()[:, k * TPS:(k + 1) * TPS],
                    in_=probs[:, k * TPS:(k + 1) * TPS])

    nc.compile()
    return nc


_NC_CACHE = None


def _get_nc():
    global _NC_CACHE
    if _NC_CACHE is None:
        _NC_CACHE = build_kernel()
    return _NC_CACHE


def _ref_probs(collocation, W, idx):
    """Exact numpy fallback for slot-overflow samples (normally none)."""
    if len(idx) == 0:
        return np.zeros(0, dtype=np.float32)
    b = collocation[idx, 1].astype(np.int64) + OFFSET + 1
    z = W[collocation[idx, 0].astype(np.int64) + OFFSET]
    levels = np.arange(DEPTH + 1)
    path = (b[:, None] >> (DEPTH - levels)) - 1
    logits = np.einsum('bpd,bd->bp', W[path], z)
    return np.prod(1.0 / (1.0 + np.exp(-logits)), axis=-1).astype(np.float32)


def _prep(collocation, W):
    """Sort, build per-tile tables/masks, pre-gather rows. Returns
    (in_maps, order, fallback_sorted_positions)."""
    Wb = W.astype(bfnp)
    ctx = collocation[:, 1].astype(np.int64)
    z0 = collocation[:, 0].astype(np.int64)
    order = np.argsort(ctx, kind="stable")
    ctx_s = ctx[order]
    z0_s = z0[order]
    b = ctx_s + (1 << DEPTH)                     # 1-based leaf ids, sorted

    ntile = BATCH // P                           # 512
    tstart = np.arange(0, BATCH, P)

    nodes = np.zeros((ntile, NCOL), dtype=np.int64)
    M = np.zeros((BATCH, NCOL), dtype=bfnp)
    fallback = []
    rows = np.arange(BATCH)
    for lev in range(NLEV_TAB):
        a = b >> (DEPTH - lev)                   # 1-based ancestor ids
        ch = np.empty(BATCH, dtype=bool)
        ch[0] = True
        ch[1:] = a[1:] != a[:-1]
        ch[tstart] = True
        cs = np.cumsum(ch)
        slot = cs - np.repeat(cs[tstart], P)     # 0-based rank within tile
        ok = slot < SLOTS[lev]
        if not ok.all():
            fallback.append(rows[~ok])
        M[rows[ok], SEG_OFF[lev] + slot[ok]] = 1
        u = np.nonzero(ch)[0]
        uk = slot[u]
        uok = uk < SLOTS[lev]
        nodes[u[uok] >> 7, SEG_OFF[lev] + uk[uok]] = a[u[uok]] - 1

    Tb = Wb[nodes]                               # [ntile, NCOL, D]
    Z = Wb[z0_s + OFFSET]                        # [BATCH, D]
    R = np.empty((BATCH, NLEV_LEAF, D), dtype=bfnp)
    for i, lev in enumerate(range(NLEV_TAB, DEPTH + 1)):
        R[:, i, :] = Wb[(b >> (DEPTH - lev)) - 1]

    in_maps = []
    for c in range(N_CORES):
        m = {}
        for k in range(SPC):
            s = SPC * c + k
            sl = slice(SH * s, SH * (s + 1))
            z3 = Z[sl].reshape(TPS, P, D)
            m[f"zp_{k}"] = np.ascontiguousarray(
                z3.transpose(1, 0, 2)).reshape(P, TPS * D)
            m[f"zt_{k}"] = np.ascontiguousarray(
                z3.transpose(2, 0, 1)).reshape(P, TPS * P)
            m[f"rl_{k}"] = np.ascontiguousarray(
                R[sl].reshape(TPS, P, NLEV_LEAF * D).transpose(1, 0, 2)
            ).reshape(P, TPS * NLEV_LEAF * D)
            m[f"mk_{k}"] = np.ascontiguousarray(
                M[sl].reshape(TPS, P, NCOL).transpose(1, 0, 2)
            ).reshape(P, TPS * NCOL)
            m[f"tb_{k}"] = np.ascontiguousarray(
                Tb[TPS * s:TPS * (s + 1)].transpose(2, 0, 1)
            ).reshape(P, TPS * NCOL)
        in_maps.append(m)

    fb = (np.unique(np.concatenate(fallback)) if fallback
          else np.zeros(0, dtype=np.int64))
    return in_maps, order, fb


def _run(collocation: np.ndarray, W: np.ndarray, trace: bool = False,
         **spmd_kwargs):
    collocation = np.ascontiguousarray(collocation, dtype=np.int32)
    W = np.ascontiguousarray(W, dtype=np.float32)
    assert collocation.shape == (BATCH, 2)
    assert W.shape == ((1 << (DEPTH + 1)) - 1, D)

    nc = _get_nc()
    in_maps, order, fb = _prep(collocation, W)

    res = run_bass_kernel_spmd(
        nc, in_maps, core_ids=list(range(N_CORES)), trace=trace,
        **spmd_kwargs)

    out = np.empty(BATCH, dtype=np.float32)
    for c in range(N_CORES):
        oc = res.results[c]["out"]               # [128, 64]
        for k in range(SPC):
            s = SPC * c + k
            vals = oc[:, k * TPS:(k + 1) * TPS].T.reshape(SH)
            out[order[SH * s:SH * (s + 1)]] = vals
    if len(fb):
        oi = order[fb]
        out[oi] = _ref_probs(collocation, W, oi)
    return out, res


def kernel(collocation: np.ndarray, W: np.ndarray) -> np.ndarray:
    out, _ = _run(collocation, W, trace=False)
    return out


# revision 10
# speedup vs baseline: 1.0438x; 1.0438x over previous
"""v16: v14 + NCOL 116, l0-8 pair-add, cross-chunk tail merge, ACT
softplus-accum product (prob = exp(-sum softplus(-logit))).

Host (data layout only, no FLOPs):
  Sort samples by context; 32 equal shards of 2048 (16 tiles of 128), core c
  runs shards 4c..4c+3.  For levels 0..14 the 128 sorted samples of a tile
  share few distinct ancestors, so each tile gets a node table (116 slot
  columns, per-level segment widths SLOTS) plus a per-sample one-hot mask.
  Leaf levels 15..20 rows and the z rows are pre-gathered per sample.

Device per shard (2048 samples):
  TensorE: psum[s, 116] = zt_tile^T @ table_tile   (dots vs all slot rows)
  ACT:     evacuate psum -> bf16
  DVE:     mm = ev * mask ; l0-8 pair-add ; l9-14 segmented reduce
           prod = rows * z ; halving tree (w16 merged across chunks)
  ACT:     nl[t] = sum softplus(-logits) ; probs = exp(-nl)
"""

import sys

for _p in ("/opt/trn_rl_repo", "/root/.axon_site/_ro/trn_rl_repo"):
    if _p not in sys.path:
        sys.path.append(_p)

import ml_dtypes
import numpy as np

import concourse.bacc as bacc
import concourse.mybir as mybir
import concourse.tile as tile
from concourse.bass_utils import run_bass_kernel_spmd

N_CORES = 8
BATCH = 65536
DEPTH = 20
OFFSET = (1 << DEPTH) - 1
D = 128
P = 128

SPC = 4                     # shards per core
NSHARD = N_CORES * SPC      # 32
SH = BATCH // NSHARD        # 2048 samples per shard
TPS = SH // P               # 16 tiles per shard
NLEV_TAB = 15               # levels 0..14 via per-tile tables
NLEV_LEAF = 6               # levels 15..20 pre-gathered rows
TPCH = 4                    # tiles per leaf chunk
NCHUNK = TPS // TPCH        # 4
NGRP = 4                    # psum groups per shard (4 tiles each)
TPG = TPS // NGRP
# (softplus not in act tables; sigmoid + bf16 product tree instead)

SLOTS = [2] * 9 + [4, 6, 8, 12, 24, 44]
SEG_OFF = np.concatenate(([0], np.cumsum(SLOTS))).astype(np.int64)
NCOL = int(SEG_OFF[-1])     # 116

f32 = mybir.dt.float32
bf16 = mybir.dt.bfloat16
bfnp = ml_dtypes.bfloat16


def build_kernel():
    nc = bacc.Bacc("TRN2", target_bir_lowering=False, debug=False,
                   num_devices=N_CORES)

    ins = []
    for k in range(SPC):
        ins.append({
            "zp": nc.dram_tensor(f"zp_{k}", [P, TPS * D], bf16,
                                 kind="ExternalInput"),
            "zt": nc.dram_tensor(f"zt_{k}", [P, TPS * P], bf16,
                                 kind="ExternalInput"),
            "rl": nc.dram_tensor(f"rl_{k}", [P, TPS * NLEV_LEAF * D], bf16,
                                 kind="ExternalInput"),
            "mk": nc.dram_tensor(f"mk_{k}", [P, TPS * NCOL], bf16,
                                 kind="ExternalInput"),
            "tb": nc.dram_tensor(f"tb_{k}", [P, TPS * NCOL], bf16,
                                 kind="ExternalInput"),
        })
    out = nc.dram_tensor("out", [P, SPC * TPS], f32, kind="ExternalOutput")

    with tile.TileContext(nc) as tc:
        with (
            tc.tile_pool(name="const", bufs=1) as cpool,
            tc.tile_pool(name="stream", bufs=2) as spool,
            tc.tile_pool(name="evp", bufs=2) as epool,
            tc.tile_pool(name="mmp", bufs=2) as wpool,
            tc.tile_pool(name="prodp", bufs=2) as ppool,
            tc.tile_pool(name="halfp", bufs=1) as hpool,
            tc.tile_pool(name="logp", bufs=2) as lpool,
            tc.tile_pool(name="psum", bufs=4, space="PSUM") as qpool,
        ):
            probs = cpool.tile([P, SPC * TPS], f32)

            for k in range(SPC):
                t_in = ins[k]
                zt = spool.tile([P, TPS, P], bf16, tag="zt")
                nc.sync.dma_start(out=zt[:], in_=t_in["zt"].ap().rearrange(
                    "p (t s) -> p t s", s=P))
                tb = spool.tile([P, TPS, NCOL], bf16, tag="tb")
                nc.sync.dma_start(out=tb[:], in_=t_in["tb"].ap().rearrange(
                    "p (t c) -> p t c", c=NCOL))
                mk = spool.tile([P, TPS, NCOL], bf16, tag="mk")
                nc.sync.dma_start(out=mk[:], in_=t_in["mk"].ap().rearrange(
                    "p (t c) -> p t c", c=NCOL))
                zp = spool.tile([P, TPS, D], bf16, tag="zp")
                nc.sync.dma_start(out=zp[:], in_=t_in["zp"].ap().rearrange(
                    "p (t d) -> p t d", d=D))
                rl = spool.tile([P, TPS, NLEV_LEAF * D], bf16, tag="rl")
                nc.sync.dma_start(out=rl[:], in_=t_in["rl"].ap().rearrange(
                    "p (t x) -> p t x", x=NLEV_LEAF * D))

                lg = lpool.tile([P, TPS, 21], f32, tag="lg")

                # --- tabled levels 0..14: matmul -> psum -> bf16 evac ---
                ev = epool.tile([P, TPS, NCOL], bf16, tag="ev")
                for g in range(NGRP):
                    pt = qpool.tile([P, TPG, NCOL], f32, tag="pt")
                    for i in range(TPG):
                        t = g * TPG + i
                        nc.tensor.matmul(pt[:, i, :], zt[:, t, :], tb[:, t, :],
                                         start=True, stop=True)
                    nc.scalar.copy(out=ev[:, g * TPG:(g + 1) * TPG, :],
                                   in_=pt[:])

                mm = wpool.tile([P, TPS, NCOL], bf16, tag="mm")
                nc.vector.tensor_tensor(out=mm[:], in0=ev[:], in1=mk[:],
                                        op=mybir.AluOpType.mult)
                # levels 0..8: nine 2-wide segments, one pair-add
                m2 = mm[:, :, 0:18].rearrange("p t (l r) -> p t l r", r=2)
                nc.vector.tensor_tensor(
                    out=lg[:, :, 0:9], in0=m2[:, :, :, 0], in1=m2[:, :, :, 1],
                    op=mybir.AluOpType.add)
                for lev in range(9, NLEV_TAB):
                    off = int(SEG_OFF[lev])
                    w = SLOTS[lev]
                    nc.vector.tensor_reduce(
                        out=lg[:, :, lev],
                        in_=mm[:, :, off:off + w],
                        axis=mybir.AxisListType.X, op=mybir.AluOpType.add)

                # --- leaf levels 15..20: per-sample dot via mult + tree ---
                p16 = hpool.tile([P, TPS, NLEV_LEAF, 16], bf16, tag="p16")
                for j in range(NCHUNK):
                    ts = j * TPCH
                    prod = ppool.tile([P, TPCH, NLEV_LEAF, D], bf16,
                                      tag="prod")
                    zc = zp[:, ts:ts + TPCH, :].unsqueeze(2)
                    nc.vector.tensor_tensor(
                        out=prod[:],
                        in0=zc.to_broadcast([P, TPCH, NLEV_LEAF, D]),
                        in1=rl[:, ts:ts + TPCH, :].rearrange(
                            "p t (l d) -> p t l d", d=D),
                        op=mybir.AluOpType.mult)
                    cur, width = prod, D
                    while width > 32:
                        width //= 2
                        nxt = hpool.tile([P, TPCH, NLEV_LEAF, width], bf16,
                                         tag=f"ph{width}", name=f"ph{width}")
                        nc.vector.tensor_tensor(
                            out=nxt[:], in0=cur[:, :, :, 0:width],
                            in1=cur[:, :, :, width:2 * width],
                            op=mybir.AluOpType.add)
                        cur = nxt
                    nc.vector.tensor_tensor(
                        out=p16[:, ts:ts + TPCH], in0=cur[:, :, :, 0:16],
                        in1=cur[:, :, :, 16:32],
                        op=mybir.AluOpType.add)
                cur, width = p16, 16
                while width > 2:
                    width //= 2
                    nxt = hpool.tile([P, TPS, NLEV_LEAF, width], bf16,
                                     tag=f"pw{width}", name=f"pw{width}")
                    nc.vector.tensor_tensor(
                        out=nxt[:], in0=cur[:, :, :, 0:width],
                        in1=cur[:, :, :, width:2 * width],
                        op=mybir.AluOpType.add)
                    cur = nxt
                nc.vector.tensor_tensor(
                    out=lg[:, :, NLEV_TAB:21],
                    in0=cur[:, :, :, 0], in1=cur[:, :, :, 1],
                    op=mybir.AluOpType.add)

                sg = lpool.tile([P, TPS, 32], f32, tag="sg")
                nc.vector.memset(sg[:, :, 21:32], 1.0)
                nc.scalar.activation(
                    out=sg[:, :, 0:21], in_=lg[:],
                    func=mybir.ActivationFunctionType.Sigmoid)
                cur, width = sg, 32
                while width > 2:
                    width //= 2
                    nxt = lpool.tile([P, TPS, width], f32,
                                     tag=f"s{width}", name=f"s{width}")
                    nc.vector.tensor_tensor(
                        out=nxt[:], in0=cur[:, :, 0:width],
                        in1=cur[:, :, width:2 * width],
                        op=mybir.AluOpType.mult)
                    cur = nxt
                nc.vector.tensor_tensor(
                    out=probs[:, k * TPS:(k + 1) * TPS], in0=cur[:, :, 0],
                    in1=cur[:, :, 1], op=mybir.AluOpType.mult)
                nc.sync.dma_start(
                    out=out.ap()[:, k * TPS:(k + 1) * TPS],
                    in_=probs[:, k * TPS:(k + 1) * TPS])

    nc.compile()
    return nc


_NC_CACHE = None


def _get_nc():
    global _NC_CACHE
    if _NC_CACHE is None:
        _NC_CACHE = build_kernel()
    return _NC_CACHE


def _ref_probs(collocation, W, idx):
    """Exact numpy fallback for slot-overflow samples (normally none)."""
    if len(idx) == 0:
        return np.zeros(0, dtype=np.float32)
    b = collocation[idx, 1].astype(np.int64) + OFFSET + 1
    z = W[collocation[idx, 0].astype(np.int64) + OFFSET]
    levels = np.arange(DEPTH + 1)
    path = (b[:, None] >> (DEPTH - levels)) - 1
    logits = np.einsum('bpd,bd->bp', W[path], z)
    return np.prod(1.0 / (1.0 + np.exp(-logits)), axis=-1).astype(np.float32)


def _prep(collocation, W):
    """Sort, build per-tile tables/masks, pre-gather rows. Returns
    (in_maps, order, fallback_sorted_positions)."""
    Wb = W.astype(bfnp)
    ctx = collocation[:, 1].astype(np.int64)
    z0 = collocation[:, 0].astype(np.int64)
    order = np.argsort(ctx, kind="stable")
    ctx_s = ctx[order]
    z0_s = z0[order]
    b = ctx_s + (1 << DEPTH)                     # 1-based leaf ids, sorted

    ntile = BATCH // P                           # 512
    tstart = np.arange(0, BATCH, P)

    nodes = np.zeros((ntile, NCOL), dtype=np.int64)
    M = np.zeros((BATCH, NCOL), dtype=bfnp)
    fallback = []
    rows = np.arange(BATCH)
    for lev in range(NLEV_TAB):
        a = b >> (DEPTH - lev)                   # 1-based ancestor ids
        ch = np.empty(BATCH, dtype=bool)
        ch[0] = True
        ch[1:] = a[1:] != a[:-1]
        ch[tstart] = True
        cs = np.cumsum(ch)
        slot = cs - np.repeat(cs[tstart], P)     # 0-based rank within tile
        ok = slot < SLOTS[lev]
        if not ok.all():
            fallback.append(rows[~ok])
        M[rows[ok], SEG_OFF[lev] + slot[ok]] = 1
        u = np.nonzero(ch)[0]
        uk = slot[u]
        uok = uk < SLOTS[lev]
        nodes[u[uok] >> 7, SEG_OFF[lev] + uk[uok]] = a[u[uok]] - 1

    Tb = Wb[nodes]                               # [ntile, NCOL, D]
    Z = Wb[z0_s + OFFSET]                        # [BATCH, D]
    R = np.empty((BATCH, NLEV_LEAF, D), dtype=bfnp)
    for i, lev in enumerate(range(NLEV_TAB, DEPTH + 1)):
        R[:, i, :] = Wb[(b >> (DEPTH - lev)) - 1]

    in_maps = []
    for c in range(N_CORES):
        m = {}
        for k in range(SPC):
            s = SPC * c + k
            sl = slice(SH * s, SH * (s + 1))
            z3 = Z[sl].reshape(TPS, P, D)
            m[f"zp_{k}"] = np.ascontiguousarray(
                z3.transpose(1, 0, 2)).reshape(P, TPS * D)
            m[f"zt_{k}"] = np.ascontiguousarray(
                z3.transpose(2, 0, 1)).reshape(P, TPS * P)
            m[f"rl_{k}"] = np.ascontiguousarray(
                R[sl].reshape(TPS, P, NLEV_LEAF * D).transpose(1, 0, 2)
            ).reshape(P, TPS * NLEV_LEAF * D)
            m[f"mk_{k}"] = np.ascontiguousarray(
                M[sl].reshape(TPS, P, NCOL).transpose(1, 0, 2)
            ).reshape(P, TPS * NCOL)
            m[f"tb_{k}"] = np.ascontiguousarray(
                Tb[TPS * s:TPS * (s + 1)].transpose(2, 0, 1)
            ).reshape(P, TPS * NCOL)
        in_maps.append(m)

    fb = (np.unique(np.concatenate(fallback)) if fallback
          else np.zeros(0, dtype=np.int64))
    return in_maps, order, fb


def _run(collocation: np.ndarray, W: np.ndarray, trace: bool = False,
         **spmd_kwargs):
    collocation = np.ascontiguousarray(collocation, dtype=np.int32)
    W = np.ascontiguousarray(W, dtype=np.float32)
    assert collocation.shape == (BATCH, 2)
    assert W.shape == ((1 << (DEPTH + 1)) - 1, D)

    nc = _get_nc()
    in_maps, order, fb = _prep(collocation, W)

    res = run_bass_kernel_spmd(
        nc, in_maps, core_ids=list(range(N_CORES)), trace=trace,
        **spmd_kwargs)

    out = np.empty(BATCH, dtype=np.float32)
    for c in range(N_CORES):
        oc = res.results[c]["out"]               # [128, 64]
        for k in range(SPC):
            s = SPC * c + k
            vals = oc[:, k * TPS:(k + 1) * TPS].T.reshape(SH)
            out[order[SH * s:SH * (s + 1)]] = vals
    if len(fb):
        oi = order[fb]
        out[oi] = _ref_probs(collocation, W, oi)
    return out, res


def kernel(collocation: np.ndarray, W: np.ndarray) -> np.ndarray:
    out, _ = _run(collocation, W, trace=False)
    return out


# revision 18
# speedup vs baseline: 1.1716x; 1.1224x over previous
"""v23: v19 + level 16 tabled (104 slots), 4 leaf levels.

Base v14: host pre-gather + per-tile ancestor tables; no device-side gathers.

Host (data layout only, no FLOPs):
  Sort samples by context; 32 equal shards of 2048 (16 tiles of 128), core c
  runs shards 4c..4c+3.  For levels 0..14 the 128 sorted samples of a tile
  share few distinct ancestors, so each tile gets a node table (124 slot
  columns, per-level segment widths SLOTS) plus a per-sample one-hot mask.
  Leaf levels 15..20 rows and the z rows are pre-gathered per sample.

Device per shard (2048 samples):
  TensorE: psum[s, 124] = zt_tile^T @ table_tile   (dots vs all slot rows)
  ACT:     evacuate psum -> bf16
  DVE:     mm = ev * mask ; per-level segmented tensor_reduce -> logits 0..14
           prod = rows * z ; halving tree -> logits 15..20
  ACT:     sigmoid ; DVE: product tree -> probs
"""

import sys

for _p in ("/opt/trn_rl_repo", "/root/.axon_site/_ro/trn_rl_repo"):
    if _p not in sys.path:
        sys.path.append(_p)

import ml_dtypes
import numpy as np

import concourse.bacc as bacc
import concourse.mybir as mybir
import concourse.tile as tile
from concourse.bass_utils import run_bass_kernel_spmd

N_CORES = 8
BATCH = 65536
DEPTH = 20
OFFSET = (1 << DEPTH) - 1
D = 128
P = 128

SPC = 4                     # shards per core
NSHARD = N_CORES * SPC      # 32
SH = BATCH // NSHARD        # 2048 samples per shard
TPS = SH // P               # 16 tiles per shard
NLEV_TAB = 17               # levels 0..16 via per-tile tables
NLEV_LEAF = 4               # levels 17..20 pre-gathered rows
TPCH = 4                    # tiles per leaf chunk
NCHUNK = TPS // TPCH        # 4
NGRP = 16                   # psum groups per shard (1 tile each)
TPG = TPS // NGRP

# slot widths per tabled level (0..8 get 2 each; deeper levels need more)
SLOTS = [2] * 9 + [4, 6, 8, 12, 24, 44, 72, 104]
SEG_OFF = np.concatenate(([0], np.cumsum(SLOTS))).astype(np.int64)
NCOL = int(SEG_OFF[-1])     # 292
HNC = NCOL // 2             # 94

f32 = mybir.dt.float32
bf16 = mybir.dt.bfloat16
bfnp = ml_dtypes.bfloat16


def build_kernel():
    nc = bacc.Bacc("TRN2", target_bir_lowering=False, debug=False,
                   num_devices=N_CORES)

    ins = []
    for k in range(SPC):
        ins.append({
            "zp": nc.dram_tensor(f"zp_{k}", [P, TPS * D], bf16,
                                 kind="ExternalInput"),
            "zt": nc.dram_tensor(f"zt_{k}", [P, TPS * P], bf16,
                                 kind="ExternalInput"),
            "rl": nc.dram_tensor(f"rl_{k}", [P, TPS * NLEV_LEAF * D], bf16,
                                 kind="ExternalInput"),
            "mk": nc.dram_tensor(f"mk_{k}", [P, TPS * NCOL], bf16,
                                 kind="ExternalInput"),
            "tb": nc.dram_tensor(f"tb_{k}", [P, TPS * NCOL], bf16,
                                 kind="ExternalInput"),
        })
    out = nc.dram_tensor("out", [P, SPC * TPS], f32, kind="ExternalOutput")

    with tile.TileContext(nc) as tc:
        with (
            tc.tile_pool(name="const", bufs=1) as cpool,
            tc.tile_pool(name="stream", bufs=2) as spool,
            tc.tile_pool(name="evp", bufs=2) as epool,
            tc.tile_pool(name="mmp", bufs=2) as wpool,
            tc.tile_pool(name="prodp", bufs=2) as ppool,
            tc.tile_pool(name="halfp", bufs=1) as hpool,
            tc.tile_pool(name="logp", bufs=2) as lpool,
            tc.tile_pool(name="psum", bufs=4, space="PSUM") as qpool,
        ):
            probs = cpool.tile([P, SPC * TPS], f32)

            for k in range(SPC):
                t_in = ins[k]
                zt = spool.tile([P, TPS, P], bf16, tag="zt")
                nc.sync.dma_start(out=zt[:], in_=t_in["zt"].ap().rearrange(
                    "p (t s) -> p t s", s=P))
                tb = spool.tile([P, TPS, NCOL], bf16, tag="tb")
                nc.sync.dma_start(out=tb[:], in_=t_in["tb"].ap().rearrange(
                    "p (t c) -> p t c", c=NCOL))
                mk = spool.tile([P, TPS, NCOL], bf16, tag="mk")
                nc.sync.dma_start(out=mk[:], in_=t_in["mk"].ap().rearrange(
                    "p (t c) -> p t c", c=NCOL))
                zp = spool.tile([P, TPS, D], bf16, tag="zp")
                nc.sync.dma_start(out=zp[:], in_=t_in["zp"].ap().rearrange(
                    "p (t d) -> p t d", d=D))
                rl = spool.tile([P, TPS, NLEV_LEAF * D], bf16, tag="rl")
                nc.sync.dma_start(out=rl[:], in_=t_in["rl"].ap().rearrange(
                    "p (t x) -> p t x", x=NLEV_LEAF * D))

                lg = lpool.tile([P, TPS, 21], f32, tag="lg")

                # --- tabled levels 0..14: matmul -> psum -> bf16 evac ---
                ev = epool.tile([P, TPS, NCOL], bf16, tag="ev")
                for g in range(NGRP):
                    pt = qpool.tile([P, TPG, NCOL], f32, tag="pt")
                    for i in range(TPG):
                        t = g * TPG + i
                        nc.tensor.matmul(pt[:, i, :], zt[:, t, :], tb[:, t, :],
                                         start=True, stop=True)
                    nc.scalar.copy(out=ev[:, g * TPG:(g + 1) * TPG, :],
                                   in_=pt[:])

                mm = wpool.tile([P, TPS, NCOL], bf16, tag="mm")
                nc.vector.tensor_tensor(out=mm[:], in0=ev[:], in1=mk[:],
                                        op=mybir.AluOpType.mult)
                # pair-split fold: slot pair members sit HNC apart (2x mode)
                p1 = wpool.tile([P, TPS, HNC], bf16, tag="p1")
                nc.vector.tensor_tensor(
                    out=p1[:], in0=mm[:, :, 0:HNC], in1=mm[:, :, HNC:NCOL],
                    op=mybir.AluOpType.add)
                # levels 0..8 fall out directly; ACT casts bf16 -> f32
                nc.scalar.copy(out=lg[:, :, 0:9], in_=p1[:, :, 0:9])
                for lev in range(9, NLEV_TAB):
                    off = int(SEG_OFF[lev]) // 2
                    w = SLOTS[lev] // 2
                    nc.vector.tensor_reduce(
                        out=lg[:, :, lev],
                        in_=p1[:, :, off:off + w],
                        axis=mybir.AxisListType.X, op=mybir.AluOpType.add)

                # --- leaf levels 15..20: per-sample dot via mult + tree ---
                for j in range(NCHUNK):
                    ts = j * TPCH
                    prod = ppool.tile([P, TPCH, NLEV_LEAF, D], bf16,
                                      tag="prod")
                    zc = zp[:, ts:ts + TPCH, :].unsqueeze(2)
                    nc.vector.tensor_tensor(
                        out=prod[:],
                        in0=zc.to_broadcast([P, TPCH, NLEV_LEAF, D]),
                        in1=rl[:, ts:ts + TPCH, :].rearrange(
                            "p t (l d) -> p t l d", d=D),
                        op=mybir.AluOpType.mult)
                    cur, width = prod, D
                    while width > 2:
                        width //= 2
                        nxt = hpool.tile([P, TPCH, NLEV_LEAF, width], bf16,
                                         tag=f"ph{width}", name=f"ph{width}")
                        nc.vector.tensor_tensor(
                            out=nxt[:], in0=cur[:, :, :, 0:width],
                            in1=cur[:, :, :, width:2 * width],
                            op=mybir.AluOpType.add)
                        cur = nxt
                    nc.vector.tensor_tensor(
                        out=lg[:, ts:ts + TPCH, NLEV_TAB:21],
                        in0=cur[:, :, :, 0], in1=cur[:, :, :, 1],
                        op=mybir.AluOpType.add)

                # --- sigmoid + product ---
                sg = lpool.tile([P, TPS, 32], f32, tag="sg")
                nc.vector.memset(sg[:, :, 21:32], 1.0)
                nc.scalar.activation(
                    out=sg[:, :, 0:21], in_=lg[:],
                    func=mybir.ActivationFunctionType.Sigmoid)
                cur, width = sg, 32
                while width > 2:
                    width //= 2
                    nxt = lpool.tile([P, TPS, width], f32, tag=f"s{width}",
                                     name=f"s{width}")
                    nc.vector.tensor_tensor(
                        out=nxt[:], in0=cur[:, :, 0:width],
                        in1=cur[:, :, width:2 * width],
                        op=mybir.AluOpType.mult)
                    cur = nxt
                nc.vector.tensor_tensor(
                    out=probs[:, k * TPS:(k + 1) * TPS], in0=cur[:, :, 0],
                    in1=cur[:, :, 1], op=mybir.AluOpType.mult)
                nc.sync.dma_start(
                    out=out.ap()[:, k * TPS:(k + 1) * TPS],
                    in_=probs[:, k * TPS:(k + 1) * TPS])

    nc.compile()
    return nc


_NC_CACHE = None


def _get_nc():
    global _NC_CACHE
    if _NC_CACHE is None:
        _NC_CACHE = build_kernel()
    return _NC_CACHE


def _ref_probs(collocation, W, idx):
    """Exact numpy fallback for slot-overflow samples (normally none)."""
    if len(idx) == 0:
        return np.zeros(0, dtype=np.float32)
    b = collocation[idx, 1].astype(np.int64) + OFFSET + 1
    z = W[collocation[idx, 0].astype(np.int64) + OFFSET]
    levels = np.arange(DEPTH + 1)
    path = (b[:, None] >> (DEPTH - levels)) - 1
    logits = np.einsum('bpd,bd->bp', W[path], z)
    return np.prod(1.0 / (1.0 + np.exp(-logits)), axis=-1).astype(np.float32)


def _prep(collocation, W):
    """Sort, build per-tile tables/masks, pre-gather rows. Returns
    (in_maps, order, fallback_original_indices)."""
    Wb = W.astype(bfnp)
    ctx = collocation[:, 1].astype(np.int64)
    z0 = collocation[:, 0].astype(np.int64)
    order = np.argsort(ctx, kind="stable")
    ctx_s = ctx[order]
    z0_s = z0[order]
    b = ctx_s + (1 << DEPTH)                     # 1-based leaf ids, sorted

    ntile = BATCH // P                           # 512
    tstart = np.arange(0, BATCH, P)

    nodes = np.zeros((ntile, NCOL), dtype=np.int64)
    M = np.zeros((BATCH, NCOL), dtype=bfnp)
    fallback = []
    rows = np.arange(BATCH)
    for lev in range(NLEV_TAB):
        a = b >> (DEPTH - lev)                   # 1-based ancestor ids
        ch = np.empty(BATCH, dtype=bool)
        ch[0] = True
        ch[1:] = a[1:] != a[:-1]
        ch[tstart] = True
        cs = np.cumsum(ch)
        slot = cs - np.repeat(cs[tstart], P)     # 0-based rank within tile
        ok = slot < SLOTS[lev]
        if not ok.all():
            fallback.append(rows[~ok])
        M[rows[ok], SEG_OFF[lev] + slot[ok]] = 1
        u = np.nonzero(ch)[0]
        uk = slot[u]
        uok = uk < SLOTS[lev]
        nodes[u[uok] >> 7, SEG_OFF[lev] + uk[uok]] = a[u[uok]] - 1

    # pair-split permutation: slot 2i -> off/2+i, slot 2i+1 -> HNC+off/2+i
    newpos = np.empty(NCOL, dtype=np.int64)
    for lev in range(NLEV_TAB):
        off = int(SEG_OFF[lev])
        w = SLOTS[lev]
        i = np.arange(w // 2)
        newpos[off + 2 * i] = off // 2 + i
        newpos[off + 2 * i + 1] = HNC + off // 2 + i
    nodes = nodes[:, np.argsort(newpos)]
    M = M[:, np.argsort(newpos)]

    Tb = Wb[nodes]                               # [ntile, NCOL, D]
    Z = Wb[z0_s + OFFSET]                        # [BATCH, D]
    R = np.empty((BATCH, NLEV_LEAF, D), dtype=bfnp)
    for i, lev in enumerate(range(NLEV_TAB, DEPTH + 1)):
        R[:, i, :] = Wb[(b >> (DEPTH - lev)) - 1]

    in_maps = []
    for c in range(N_CORES):
        m = {}
        for k in range(SPC):
            s = SPC * c + k
            sl = slice(SH * s, SH * (s + 1))
            z3 = Z[sl].reshape(TPS, P, D)
            m[f"zp_{k}"] = np.ascontiguousarray(
                z3.transpose(1, 0, 2)).reshape(P, TPS * D)
            m[f"zt_{k}"] = np.ascontiguousarray(
                z3.transpose(2, 0, 1)).reshape(P, TPS * P)
            m[f"rl_{k}"] = np.ascontiguousarray(
                R[sl].reshape(TPS, P, NLEV_LEAF * D).transpose(1, 0, 2)
            ).reshape(P, TPS * NLEV_LEAF * D)
            m[f"mk_{k}"] = np.ascontiguousarray(
                M[sl].reshape(TPS, P, NCOL).transpose(1, 0, 2)
            ).reshape(P, TPS * NCOL)
            m[f"tb_{k}"] = np.ascontiguousarray(
                Tb[TPS * s:TPS * (s + 1)].transpose(2, 0, 1)
            ).reshape(P, TPS * NCOL)
        in_maps.append(m)

    fb = (np.unique(np.concatenate(fallback)) if fallback
          else np.zeros(0, dtype=np.int64))
    return in_maps, order, fb


def _run(collocation: np.ndarray, W: np.ndarray, trace: bool = False,
         **spmd_kwargs):
    collocation = np.ascontiguousarray(collocation, dtype=np.int32)
    W = np.ascontiguousarray(W, dtype=np.float32)
    assert collocation.shape == (BATCH, 2)
    assert W.shape == ((1 << (DEPTH + 1)) - 1, D)

    nc = _get_nc()
    in_maps, order, fb = _prep(collocation, W)

    res = run_bass_kernel_spmd(
        nc, in_maps, core_ids=list(range(N_CORES)), trace=trace,
        **spmd_kwargs)

    out = np.empty(BATCH, dtype=np.float32)
    for c in range(N_CORES):
        oc = res.results[c]["out"]               # [128, 64]
        for k in range(SPC):
            s = SPC * c + k
            vals = oc[:, k * TPS:(k + 1) * TPS].T.reshape(SH)
            out[order[SH * s:SH * (s + 1)]] = vals
    if len(fb):
        oi = order[fb]
        out[oi] = _ref_probs(collocation, W, oi)
    return out, res


def kernel(collocation: np.ndarray, W: np.ndarray) -> np.ndarray:
    out, _ = _run(collocation, W, trace=False)
    return out


# revision 19
# speedup vs baseline: 1.2583x; 1.0740x over previous
"""v24: v19 + leaf chunks of 8 tiles.

Base v14: host pre-gather + per-tile ancestor tables; no device-side gathers.

Host (data layout only, no FLOPs):
  Sort samples by context; 32 equal shards of 2048 (16 tiles of 128), core c
  runs shards 4c..4c+3.  For levels 0..14 the 128 sorted samples of a tile
  share few distinct ancestors, so each tile gets a node table (124 slot
  columns, per-level segment widths SLOTS) plus a per-sample one-hot mask.
  Leaf levels 15..20 rows and the z rows are pre-gathered per sample.

Device per shard (2048 samples):
  TensorE: psum[s, 124] = zt_tile^T @ table_tile   (dots vs all slot rows)
  ACT:     evacuate psum -> bf16
  DVE:     mm = ev * mask ; per-level segmented tensor_reduce -> logits 0..14
           prod = rows * z ; halving tree -> logits 15..20
  ACT:     sigmoid ; DVE: product tree -> probs
"""

import sys

for _p in ("/opt/trn_rl_repo", "/root/.axon_site/_ro/trn_rl_repo"):
    if _p not in sys.path:
        sys.path.append(_p)

import ml_dtypes
import numpy as np

import concourse.bacc as bacc
import concourse.mybir as mybir
import concourse.tile as tile
from concourse.bass_utils import run_bass_kernel_spmd

N_CORES = 8
BATCH = 65536
DEPTH = 20
OFFSET = (1 << DEPTH) - 1
D = 128
P = 128

SPC = 4                     # shards per core
NSHARD = N_CORES * SPC      # 32
SH = BATCH // NSHARD        # 2048 samples per shard
TPS = SH // P               # 16 tiles per shard
NLEV_TAB = 16               # levels 0..15 via per-tile tables
NLEV_LEAF = 5               # levels 16..20 pre-gathered rows
TPCH = 8                    # tiles per leaf chunk
NCHUNK = TPS // TPCH        # 4
NGRP = 8                    # psum groups per shard (2 tiles each)
TPG = TPS // NGRP

# slot widths per tabled level (0..8 get 2 each; deeper levels need more)
SLOTS = [2] * 9 + [4, 6, 8, 12, 24, 44, 72]
SEG_OFF = np.concatenate(([0], np.cumsum(SLOTS))).astype(np.int64)
NCOL = int(SEG_OFF[-1])     # 188
HNC = NCOL // 2             # 94

f32 = mybir.dt.float32
bf16 = mybir.dt.bfloat16
bfnp = ml_dtypes.bfloat16


def build_kernel():
    nc = bacc.Bacc("TRN2", target_bir_lowering=False, debug=False,
                   num_devices=N_CORES)

    ins = []
    for k in range(SPC):
        ins.append({
            "zp": nc.dram_tensor(f"zp_{k}", [P, TPS * D], bf16,
                                 kind="ExternalInput"),
            "zt": nc.dram_tensor(f"zt_{k}", [P, TPS * P], bf16,
                                 kind="ExternalInput"),
            "rl": nc.dram_tensor(f"rl_{k}", [P, TPS * NLEV_LEAF * D], bf16,
                                 kind="ExternalInput"),
            "mk": nc.dram_tensor(f"mk_{k}", [P, TPS * NCOL], bf16,
                                 kind="ExternalInput"),
            "tb": nc.dram_tensor(f"tb_{k}", [P, TPS * NCOL], bf16,
                                 kind="ExternalInput"),
        })
    out = nc.dram_tensor("out", [P, SPC * TPS], f32, kind="ExternalOutput")

    with tile.TileContext(nc) as tc:
        with (
            tc.tile_pool(name="const", bufs=1) as cpool,
            tc.tile_pool(name="stream", bufs=2) as spool,
            tc.tile_pool(name="evp", bufs=2) as epool,
            tc.tile_pool(name="mmp", bufs=2) as wpool,
            tc.tile_pool(name="prodp", bufs=2) as ppool,
            tc.tile_pool(name="halfp", bufs=1) as hpool,
            tc.tile_pool(name="logp", bufs=2) as lpool,
            tc.tile_pool(name="psum", bufs=4, space="PSUM") as qpool,
        ):
            probs = cpool.tile([P, SPC * TPS], f32)

            for k in range(SPC):
                t_in = ins[k]
                zt = spool.tile([P, TPS, P], bf16, tag="zt")
                nc.sync.dma_start(out=zt[:], in_=t_in["zt"].ap().rearrange(
                    "p (t s) -> p t s", s=P))
                tb = spool.tile([P, TPS, NCOL], bf16, tag="tb")
                nc.sync.dma_start(out=tb[:], in_=t_in["tb"].ap().rearrange(
                    "p (t c) -> p t c", c=NCOL))
                mk = spool.tile([P, TPS, NCOL], bf16, tag="mk")
                nc.sync.dma_start(out=mk[:], in_=t_in["mk"].ap().rearrange(
                    "p (t c) -> p t c", c=NCOL))
                zp = spool.tile([P, TPS, D], bf16, tag="zp")
                nc.sync.dma_start(out=zp[:], in_=t_in["zp"].ap().rearrange(
                    "p (t d) -> p t d", d=D))
                rl = spool.tile([P, TPS, NLEV_LEAF * D], bf16, tag="rl")
                nc.sync.dma_start(out=rl[:], in_=t_in["rl"].ap().rearrange(
                    "p (t x) -> p t x", x=NLEV_LEAF * D))

                lg = lpool.tile([P, TPS, 21], f32, tag="lg")

                # --- tabled levels 0..14: matmul -> psum -> bf16 evac ---
                ev = epool.tile([P, TPS, NCOL], bf16, tag="ev")
                for g in range(NGRP):
                    pt = qpool.tile([P, TPG, NCOL], f32, tag="pt")
                    for i in range(TPG):
                        t = g * TPG + i
                        nc.tensor.matmul(pt[:, i, :], zt[:, t, :], tb[:, t, :],
                                         start=True, stop=True)
                    nc.scalar.copy(out=ev[:, g * TPG:(g + 1) * TPG, :],
                                   in_=pt[:])

                mm = wpool.tile([P, TPS, NCOL], bf16, tag="mm")
                nc.vector.tensor_tensor(out=mm[:], in0=ev[:], in1=mk[:],
                                        op=mybir.AluOpType.mult)
                # pair-split fold: slot pair members sit HNC apart (2x mode)
                p1 = wpool.tile([P, TPS, HNC], bf16, tag="p1")
                nc.vector.tensor_tensor(
                    out=p1[:], in0=mm[:, :, 0:HNC], in1=mm[:, :, HNC:NCOL],
                    op=mybir.AluOpType.add)
                # levels 0..8 fall out directly; ACT casts bf16 -> f32
                nc.scalar.copy(out=lg[:, :, 0:9], in_=p1[:, :, 0:9])
                for lev in range(9, NLEV_TAB):
                    off = int(SEG_OFF[lev]) // 2
                    w = SLOTS[lev] // 2
                    nc.vector.tensor_reduce(
                        out=lg[:, :, lev],
                        in_=p1[:, :, off:off + w],
                        axis=mybir.AxisListType.X, op=mybir.AluOpType.add)

                # --- leaf levels 15..20: per-sample dot via mult + tree ---
                for j in range(NCHUNK):
                    ts = j * TPCH
                    prod = ppool.tile([P, TPCH, NLEV_LEAF, D], bf16,
                                      tag="prod")
                    zc = zp[:, ts:ts + TPCH, :].unsqueeze(2)
                    nc.vector.tensor_tensor(
                        out=prod[:],
                        in0=zc.to_broadcast([P, TPCH, NLEV_LEAF, D]),
                        in1=rl[:, ts:ts + TPCH, :].rearrange(
                            "p t (l d) -> p t l d", d=D),
                        op=mybir.AluOpType.mult)
                    cur, width = prod, D
                    while width > 2:
                        width //= 2
                        nxt = hpool.tile([P, TPCH, NLEV_LEAF, width], bf16,
                                         tag=f"ph{width}", name=f"ph{width}")
                        nc.vector.tensor_tensor(
                            out=nxt[:], in0=cur[:, :, :, 0:width],
                            in1=cur[:, :, :, width:2 * width],
                            op=mybir.AluOpType.add)
                        cur = nxt
                    nc.vector.tensor_tensor(
                        out=lg[:, ts:ts + TPCH, NLEV_TAB:21],
                        in0=cur[:, :, :, 0], in1=cur[:, :, :, 1],
                        op=mybir.AluOpType.add)

                # --- sigmoid + product ---
                sg = lpool.tile([P, TPS, 32], f32, tag="sg")
                nc.vector.memset(sg[:, :, 21:32], 1.0)
                nc.scalar.activation(
                    out=sg[:, :, 0:21], in_=lg[:],
                    func=mybir.ActivationFunctionType.Sigmoid)
                cur, width = sg, 32
                while width > 2:
                    width //= 2
                    nxt = lpool.tile([P, TPS, width], f32, tag=f"s{width}",
                                     name=f"s{width}")
                    nc.vector.tensor_tensor(
                        out=nxt[:], in0=cur[:, :, 0:width],
                        in1=cur[:, :, width:2 * width],
                        op=mybir.AluOpType.mult)
                    cur = nxt
                nc.vector.tensor_tensor(
                    out=probs[:, k * TPS:(k + 1) * TPS], in0=cur[:, :, 0],
                    in1=cur[:, :, 1], op=mybir.AluOpType.mult)
                nc.sync.dma_start(
                    out=out.ap()[:, k * TPS:(k + 1) * TPS],
                    in_=probs[:, k * TPS:(k + 1) * TPS])

    nc.compile()
    return nc


_NC_CACHE = None


def _get_nc():
    global _NC_CACHE
    if _NC_CACHE is None:
        _NC_CACHE = build_kernel()
    return _NC_CACHE


def _ref_probs(collocation, W, idx):
    """Exact numpy fallback for slot-overflow samples (normally none)."""
    if len(idx) == 0:
        return np.zeros(0, dtype=np.float32)
    b = collocation[idx, 1].astype(np.int64) + OFFSET + 1
    z = W[collocation[idx, 0].astype(np.int64) + OFFSET]
    levels = np.arange(DEPTH + 1)
    path = (b[:, None] >> (DEPTH - levels)) - 1
    logits = np.einsum('bpd,bd->bp', W[path], z)
    return np.prod(1.0 / (1.0 + np.exp(-logits)), axis=-1).astype(np.float32)


def _prep(collocation, W):
    """Sort, build per-tile tables/masks, pre-gather rows. Returns
    (in_maps, order, fallback_original_indices)."""
    Wb = W.astype(bfnp)
    ctx = collocation[:, 1].astype(np.int64)
    z0 = collocation[:, 0].astype(np.int64)
    order = np.argsort(ctx, kind="stable")
    ctx_s = ctx[order]
    z0_s = z0[order]
    b = ctx_s + (1 << DEPTH)                     # 1-based leaf ids, sorted

    ntile = BATCH // P                           # 512
    tstart = np.arange(0, BATCH, P)

    nodes = np.zeros((ntile, NCOL), dtype=np.int64)
    M = np.zeros((BATCH, NCOL), dtype=bfnp)
    fallback = []
    rows = np.arange(BATCH)
    for lev in range(NLEV_TAB):
        a = b >> (DEPTH - lev)                   # 1-based ancestor ids
        ch = np.empty(BATCH, dtype=bool)
        ch[0] = True
        ch[1:] = a[1:] != a[:-1]
        ch[tstart] = True
        cs = np.cumsum(ch)
        slot = cs - np.repeat(cs[tstart], P)     # 0-based rank within tile
        ok = slot < SLOTS[lev]
        if not ok.all():
            fallback.append(rows[~ok])
        M[rows[ok], SEG_OFF[lev] + slot[ok]] = 1
        u = np.nonzero(ch)[0]
        uk = slot[u]
        uok = uk < SLOTS[lev]
        nodes[u[uok] >> 7, SEG_OFF[lev] + uk[uok]] = a[u[uok]] - 1

    # pair-split permutation: slot 2i -> off/2+i, slot 2i+1 -> HNC+off/2+i
    newpos = np.empty(NCOL, dtype=np.int64)
    for lev in range(NLEV_TAB):
        off = int(SEG_OFF[lev])
        w = SLOTS[lev]
        i = np.arange(w // 2)
        newpos[off + 2 * i] = off // 2 + i
        newpos[off + 2 * i + 1] = HNC + off // 2 + i
    nodes = nodes[:, np.argsort(newpos)]
    M = M[:, np.argsort(newpos)]

    Tb = Wb[nodes]                               # [ntile, NCOL, D]
    Z = Wb[z0_s + OFFSET]                        # [BATCH, D]
    R = np.empty((BATCH, NLEV_LEAF, D), dtype=bfnp)
    for i, lev in enumerate(range(NLEV_TAB, DEPTH + 1)):
        R[:, i, :] = Wb[(b >> (DEPTH - lev)) - 1]

    in_maps = []
    for c in range(N_CORES):
        m = {}
        for k in range(SPC):
            s = SPC * c + k
            sl = slice(SH * s, SH * (s + 1))
            z3 = Z[sl].reshape(TPS, P, D)
            m[f"zp_{k}"] = np.ascontiguousarray(
                z3.transpose(1, 0, 2)).reshape(P, TPS * D)
            m[f"zt_{k}"] = np.ascontiguousarray(
                z3.transpose(2, 0, 1)).reshape(P, TPS * P)
            m[f"rl_{k}"] = np.ascontiguousarray(
                R[sl].reshape(TPS, P, NLEV_LEAF * D).transpose(1, 0, 2)
            ).reshape(P, TPS * NLEV_LEAF * D)
            m[f"mk_{k}"] = np.ascontiguousarray(
                M[sl].reshape(TPS, P, NCOL).transpose(1, 0, 2)
            ).reshape(P, TPS * NCOL)
            m[f"tb_{k}"] = np.ascontiguousarray(
                Tb[TPS * s:TPS * (s + 1)].transpose(2, 0, 1)
            ).reshape(P, TPS * NCOL)
        in_maps.append(m)

    fb = (np.unique(np.concatenate(fallback)) if fallback
          else np.zeros(0, dtype=np.int64))
    return in_maps, order, fb


def _run(collocation: np.ndarray, W: np.ndarray, trace: bool = False,
         **spmd_kwargs):
    collocation = np.ascontiguousarray(collocation, dtype=np.int32)
    W = np.ascontiguousarray(W, dtype=np.float32)
    assert collocation.shape == (BATCH, 2)
    assert W.shape == ((1 << (DEPTH + 1)) - 1, D)

    nc = _get_nc()
    in_maps, order, fb = _prep(collocation, W)

    res = run_bass_kernel_spmd(
        nc, in_maps, core_ids=list(range(N_CORES)), trace=trace,
        **spmd_kwargs)

    out = np.empty(BATCH, dtype=np.float32)
    for c in range(N_CORES):
        oc = res.results[c]["out"]               # [128, 64]
        for k in range(SPC):
            s = SPC * c + k
            vals = oc[:, k * TPS:(k + 1) * TPS].T.reshape(SH)
            out[order[SH * s:SH * (s + 1)]] = vals
    if len(fb):
        oi = order[fb]
        out[oi] = _ref_probs(collocation, W, oi)
    return out, res


def kernel(collocation: np.ndarray, W: np.ndarray) -> np.ndarray:
    out, _ = _run(collocation, W, trace=False)
    return out


# revision 20
# speedup vs baseline: 1.2606x; 1.0018x over previous
"""v25: v19 + single leaf chunk per shard.

Base v14: host pre-gather + per-tile ancestor tables; no device-side gathers.

Host (data layout only, no FLOPs):
  Sort samples by context; 32 equal shards of 2048 (16 tiles of 128), core c
  runs shards 4c..4c+3.  For levels 0..14 the 128 sorted samples of a tile
  share few distinct ancestors, so each tile gets a node table (124 slot
  columns, per-level segment widths SLOTS) plus a per-sample one-hot mask.
  Leaf levels 15..20 rows and the z rows are pre-gathered per sample.

Device per shard (2048 samples):
  TensorE: psum[s, 124] = zt_tile^T @ table_tile   (dots vs all slot rows)
  ACT:     evacuate psum -> bf16
  DVE:     mm = ev * mask ; per-level segmented tensor_reduce -> logits 0..14
           prod = rows * z ; halving tree -> logits 15..20
  ACT:     sigmoid ; DVE: product tree -> probs
"""

import sys

for _p in ("/opt/trn_rl_repo", "/root/.axon_site/_ro/trn_rl_repo"):
    if _p not in sys.path:
        sys.path.append(_p)

import ml_dtypes
import numpy as np

import concourse.bacc as bacc
import concourse.mybir as mybir
import concourse.tile as tile
from concourse.bass_utils import run_bass_kernel_spmd

N_CORES = 8
BATCH = 65536
DEPTH = 20
OFFSET = (1 << DEPTH) - 1
D = 128
P = 128

SPC = 4                     # shards per core
NSHARD = N_CORES * SPC      # 32
SH = BATCH // NSHARD        # 2048 samples per shard
TPS = SH // P               # 16 tiles per shard
NLEV_TAB = 16               # levels 0..15 via per-tile tables
NLEV_LEAF = 5               # levels 16..20 pre-gathered rows
TPCH = 16                   # tiles per leaf chunk
NCHUNK = TPS // TPCH        # 4
NGRP = 8                    # psum groups per shard (2 tiles each)
TPG = TPS // NGRP

# slot widths per tabled level (0..8 get 2 each; deeper levels need more)
SLOTS = [2] * 9 + [4, 6, 8, 12, 24, 44, 72]
SEG_OFF = np.concatenate(([0], np.cumsum(SLOTS))).astype(np.int64)
NCOL = int(SEG_OFF[-1])     # 188
HNC = NCOL // 2             # 94

f32 = mybir.dt.float32
bf16 = mybir.dt.bfloat16
bfnp = ml_dtypes.bfloat16


def build_kernel():
    nc = bacc.Bacc("TRN2", target_bir_lowering=False, debug=False,
                   num_devices=N_CORES)

    ins = []
    for k in range(SPC):
        ins.append({
            "zp": nc.dram_tensor(f"zp_{k}", [P, TPS * D], bf16,
                                 kind="ExternalInput"),
            "zt": nc.dram_tensor(f"zt_{k}", [P, TPS * P], bf16,
                                 kind="ExternalInput"),
            "rl": nc.dram_tensor(f"rl_{k}", [P, TPS * NLEV_LEAF * D], bf16,
                                 kind="ExternalInput"),
            "mk": nc.dram_tensor(f"mk_{k}", [P, TPS * NCOL], bf16,
                                 kind="ExternalInput"),
            "tb": nc.dram_tensor(f"tb_{k}", [P, TPS * NCOL], bf16,
                                 kind="ExternalInput"),
        })
    out = nc.dram_tensor("out", [P, SPC * TPS], f32, kind="ExternalOutput")

    with tile.TileContext(nc) as tc:
        with (
            tc.tile_pool(name="const", bufs=1) as cpool,
            tc.tile_pool(name="stream", bufs=2) as spool,
            tc.tile_pool(name="evp", bufs=2) as epool,
            tc.tile_pool(name="mmp", bufs=2) as wpool,
            tc.tile_pool(name="prodp", bufs=2) as ppool,
            tc.tile_pool(name="halfp", bufs=1) as hpool,
            tc.tile_pool(name="logp", bufs=2) as lpool,
            tc.tile_pool(name="psum", bufs=4, space="PSUM") as qpool,
        ):
            probs = cpool.tile([P, SPC * TPS], f32)

            for k in range(SPC):
                t_in = ins[k]
                zt = spool.tile([P, TPS, P], bf16, tag="zt")
                nc.sync.dma_start(out=zt[:], in_=t_in["zt"].ap().rearrange(
                    "p (t s) -> p t s", s=P))
                tb = spool.tile([P, TPS, NCOL], bf16, tag="tb")
                nc.sync.dma_start(out=tb[:], in_=t_in["tb"].ap().rearrange(
                    "p (t c) -> p t c", c=NCOL))
                mk = spool.tile([P, TPS, NCOL], bf16, tag="mk")
                nc.sync.dma_start(out=mk[:], in_=t_in["mk"].ap().rearrange(
                    "p (t c) -> p t c", c=NCOL))
                zp = spool.tile([P, TPS, D], bf16, tag="zp")
                nc.sync.dma_start(out=zp[:], in_=t_in["zp"].ap().rearrange(
                    "p (t d) -> p t d", d=D))
                rl = spool.tile([P, TPS, NLEV_LEAF * D], bf16, tag="rl")
                nc.sync.dma_start(out=rl[:], in_=t_in["rl"].ap().rearrange(
                    "p (t x) -> p t x", x=NLEV_LEAF * D))

                lg = lpool.tile([P, TPS, 21], f32, tag="lg")

                # --- tabled levels 0..14: matmul -> psum -> bf16 evac ---
                ev = epool.tile([P, TPS, NCOL], bf16, tag="ev")
                for g in range(NGRP):
                    pt = qpool.tile([P, TPG, NCOL], f32, tag="pt")
                    for i in range(TPG):
                        t = g * TPG + i
                        nc.tensor.matmul(pt[:, i, :], zt[:, t, :], tb[:, t, :],
                                         start=True, stop=True)
                    nc.scalar.copy(out=ev[:, g * TPG:(g + 1) * TPG, :],
                                   in_=pt[:])

                mm = wpool.tile([P, TPS, NCOL], bf16, tag="mm")
                nc.vector.tensor_tensor(out=mm[:], in0=ev[:], in1=mk[:],
                                        op=mybir.AluOpType.mult)
                # pair-split fold: slot pair members sit HNC apart (2x mode)
                p1 = wpool.tile([P, TPS, HNC], bf16, tag="p1")
                nc.vector.tensor_tensor(
                    out=p1[:], in0=mm[:, :, 0:HNC], in1=mm[:, :, HNC:NCOL],
                    op=mybir.AluOpType.add)
                # levels 0..8 fall out directly; ACT casts bf16 -> f32
                nc.scalar.copy(out=lg[:, :, 0:9], in_=p1[:, :, 0:9])
                for lev in range(9, NLEV_TAB):
                    off = int(SEG_OFF[lev]) // 2
                    w = SLOTS[lev] // 2
                    nc.vector.tensor_reduce(
                        out=lg[:, :, lev],
                        in_=p1[:, :, off:off + w],
                        axis=mybir.AxisListType.X, op=mybir.AluOpType.add)

                # --- leaf levels 15..20: per-sample dot via mult + tree ---
                for j in range(NCHUNK):
                    ts = j * TPCH
                    prod = ppool.tile([P, TPCH, NLEV_LEAF, D], bf16,
                                      tag="prod")
                    zc = zp[:, ts:ts + TPCH, :].unsqueeze(2)
                    nc.vector.tensor_tensor(
                        out=prod[:],
                        in0=zc.to_broadcast([P, TPCH, NLEV_LEAF, D]),
                        in1=rl[:, ts:ts + TPCH, :].rearrange(
                            "p t (l d) -> p t l d", d=D),
                        op=mybir.AluOpType.mult)
                    cur, width = prod, D
                    while width > 2:
                        width //= 2
                        nxt = hpool.tile([P, TPCH, NLEV_LEAF, width], bf16,
                                         tag=f"ph{width}", name=f"ph{width}")
                        nc.vector.tensor_tensor(
                            out=nxt[:], in0=cur[:, :, :, 0:width],
                            in1=cur[:, :, :, width:2 * width],
                            op=mybir.AluOpType.add)
                        cur = nxt
                    nc.vector.tensor_tensor(
                        out=lg[:, ts:ts + TPCH, NLEV_TAB:21],
                        in0=cur[:, :, :, 0], in1=cur[:, :, :, 1],
                        op=mybir.AluOpType.add)

                # --- sigmoid + product ---
                sg = lpool.tile([P, TPS, 32], f32, tag="sg")
                nc.vector.memset(sg[:, :, 21:32], 1.0)
                nc.scalar.activation(
                    out=sg[:, :, 0:21], in_=lg[:],
                    func=mybir.ActivationFunctionType.Sigmoid)
                cur, width = sg, 32
                while width > 2:
                    width //= 2
                    nxt = lpool.tile([P, TPS, width], f32, tag=f"s{width}",
                                     name=f"s{width}")
                    nc.vector.tensor_tensor(
                        out=nxt[:], in0=cur[:, :, 0:width],
                        in1=cur[:, :, width:2 * width],
                        op=mybir.AluOpType.mult)
                    cur = nxt
                nc.vector.tensor_tensor(
                    out=probs[:, k * TPS:(k + 1) * TPS], in0=cur[:, :, 0],
                    in1=cur[:, :, 1], op=mybir.AluOpType.mult)
                nc.sync.dma_start(
                    out=out.ap()[:, k * TPS:(k + 1) * TPS],
                    in_=probs[:, k * TPS:(k + 1) * TPS])

    nc.compile()
    return nc


_NC_CACHE = None


def _get_nc():
    global _NC_CACHE
    if _NC_CACHE is None:
        _NC_CACHE = build_kernel()
    return _NC_CACHE


def _ref_probs(collocation, W, idx):
    """Exact numpy fallback for slot-overflow samples (normally none)."""
    if len(idx) == 0:
        return np.zeros(0, dtype=np.float32)
    b = collocation[idx, 1].astype(np.int64) + OFFSET + 1
    z = W[collocation[idx, 0].astype(np.int64) + OFFSET]
    levels = np.arange(DEPTH + 1)
    path = (b[:, None] >> (DEPTH - levels)) - 1
    logits = np.einsum('bpd,bd->bp', W[path], z)
    return np.prod(1.0 / (1.0 + np.exp(-logits)), axis=-1).astype(np.float32)


def _prep(collocation, W):
    """Sort, build per-tile tables/masks, pre-gather rows. Returns
    (in_maps, order, fallback_original_indices)."""
    Wb = W.astype(bfnp)
    ctx = collocation[:, 1].astype(np.int64)
    z0 = collocation[:, 0].astype(np.int64)
    order = np.argsort(ctx, kind="stable")
    ctx_s = ctx[order]
    z0_s = z0[order]
    b = ctx_s + (1 << DEPTH)                     # 1-based leaf ids, sorted

    ntile = BATCH // P                           # 512
    tstart = np.arange(0, BATCH, P)

    nodes = np.zeros((ntile, NCOL), dtype=np.int64)
    M = np.zeros((BATCH, NCOL), dtype=bfnp)
    fallback = []
    rows = np.arange(BATCH)
    for lev in range(NLEV_TAB):
        a = b >> (DEPTH - lev)                   # 1-based ancestor ids
        ch = np.empty(BATCH, dtype=bool)
        ch[0] = True
        ch[1:] = a[1:] != a[:-1]
        ch[tstart] = True
        cs = np.cumsum(ch)
        slot = cs - np.repeat(cs[tstart], P)     # 0-based rank within tile
        ok = slot < SLOTS[lev]
        if not ok.all():
            fallback.append(rows[~ok])
        M[rows[ok], SEG_OFF[lev] + slot[ok]] = 1
        u = np.nonzero(ch)[0]
        uk = slot[u]
        uok = uk < SLOTS[lev]
        nodes[u[uok] >> 7, SEG_OFF[lev] + uk[uok]] = a[u[uok]] - 1

    # pair-split permutation: slot 2i -> off/2+i, slot 2i+1 -> HNC+off/2+i
    newpos = np.empty(NCOL, dtype=np.int64)
    for lev in range(NLEV_TAB):
        off = int(SEG_OFF[lev])
        w = SLOTS[lev]
        i = np.arange(w // 2)
        newpos[off + 2 * i] = off // 2 + i
        newpos[off + 2 * i + 1] = HNC + off // 2 + i
    nodes = nodes[:, np.argsort(newpos)]
    M = M[:, np.argsort(newpos)]

    Tb = Wb[nodes]                               # [ntile, NCOL, D]
    Z = Wb[z0_s + OFFSET]                        # [BATCH, D]
    R = np.empty((BATCH, NLEV_LEAF, D), dtype=bfnp)
    for i, lev in enumerate(range(NLEV_TAB, DEPTH + 1)):
        R[:, i, :] = Wb[(b >> (DEPTH - lev)) - 1]

    in_maps = []
    for c in range(N_CORES):
        m = {}
        for k in range(SPC):
            s = SPC * c + k
            sl = slice(SH * s, SH * (s + 1))
            z3 = Z[sl].reshape(TPS, P, D)
            m[f"zp_{k}"] = np.ascontiguousarray(
                z3.transpose(1, 0, 2)).reshape(P, TPS * D)
            m[f"zt_{k}"] = np.ascontiguousarray(
                z3.transpose(2, 0, 1)).reshape(P, TPS * P)
            m[f"rl_{k}"] = np.ascontiguousarray(
                R[sl].reshape(TPS, P, NLEV_LEAF * D).transpose(1, 0, 2)
            ).reshape(P, TPS * NLEV_LEAF * D)
            m[f"mk_{k}"] = np.ascontiguousarray(
                M[sl].reshape(TPS, P, NCOL).transpose(1, 0, 2)
            ).reshape(P, TPS * NCOL)
            m[f"tb_{k}"] = np.ascontiguousarray(
                Tb[TPS * s:TPS * (s + 1)].transpose(2, 0, 1)
            ).reshape(P, TPS * NCOL)
        in_maps.append(m)

    fb = (np.unique(np.concatenate(fallback)) if fallback
          else np.zeros(0, dtype=np.int64))
    return in_maps, order, fb


def _run(collocation: np.ndarray, W: np.ndarray, trace: bool = False,
         **spmd_kwargs):
    collocation = np.ascontiguousarray(collocation, dtype=np.int32)
    W = np.ascontiguousarray(W, dtype=np.float32)
    assert collocation.shape == (BATCH, 2)
    assert W.shape == ((1 << (DEPTH + 1)) - 1, D)

    nc = _get_nc()
    in_maps, order, fb = _prep(collocation, W)

    res = run_bass_kernel_spmd(
        nc, in_maps, core_ids=list(range(N_CORES)), trace=trace,
        **spmd_kwargs)

    out = np.empty(BATCH, dtype=np.float32)
    for c in range(N_CORES):
        oc = res.results[c]["out"]               # [128, 64]
        for k in range(SPC):
            s = SPC * c + k
            vals = oc[:, k * TPS:(k + 1) * TPS].T.reshape(SH)
            out[order[SH * s:SH * (s + 1)]] = vals
    if len(fb):
        oi = order[fb]
        out[oi] = _ref_probs(collocation, W, oi)
    return out, res


def kernel(collocation: np.ndarray, W: np.ndarray) -> np.ndarray:
    out, _ = _run(collocation, W, trace=False)
    return out


# revision 23
# speedup vs baseline: 1.2830x; 1.0177x over previous
"""v26: v25 + leaf block issued before tabled block.

Base v14: host pre-gather + per-tile ancestor tables; no device-side gathers.

Host (data layout only, no FLOPs):
  Sort samples by context; 32 equal shards of 2048 (16 tiles of 128), core c
  runs shards 4c..4c+3.  For levels 0..14 the 128 sorted samples of a tile
  share few distinct ancestors, so each tile gets a node table (124 slot
  columns, per-level segment widths SLOTS) plus a per-sample one-hot mask.
  Leaf levels 15..20 rows and the z rows are pre-gathered per sample.

Device per shard (2048 samples):
  TensorE: psum[s, 124] = zt_tile^T @ table_tile   (dots vs all slot rows)
  ACT:     evacuate psum -> bf16
  DVE:     mm = ev * mask ; per-level segmented tensor_reduce -> logits 0..14
           prod = rows * z ; halving tree -> logits 15..20
  ACT:     sigmoid ; DVE: product tree -> probs
"""

import sys

for _p in ("/opt/trn_rl_repo", "/root/.axon_site/_ro/trn_rl_repo"):
    if _p not in sys.path:
        sys.path.append(_p)

import ml_dtypes
import numpy as np

import concourse.bacc as bacc
import concourse.mybir as mybir
import concourse.tile as tile
from concourse.bass_utils import run_bass_kernel_spmd

N_CORES = 8
BATCH = 65536
DEPTH = 20
OFFSET = (1 << DEPTH) - 1
D = 128
P = 128

SPC = 4                     # shards per core
NSHARD = N_CORES * SPC      # 32
SH = BATCH // NSHARD        # 2048 samples per shard
TPS = SH // P               # 16 tiles per shard
NLEV_TAB = 16               # levels 0..15 via per-tile tables
NLEV_LEAF = 5               # levels 16..20 pre-gathered rows
TPCH = 16                   # tiles per leaf chunk
NCHUNK = TPS // TPCH        # 4
NGRP = 8                    # psum groups per shard (2 tiles each)
TPG = TPS // NGRP

# slot widths per tabled level (0..8 get 2 each; deeper levels need more)
SLOTS = [2] * 9 + [4, 6, 8, 12, 24, 44, 72]
SEG_OFF = np.concatenate(([0], np.cumsum(SLOTS))).astype(np.int64)
NCOL = int(SEG_OFF[-1])     # 188
HNC = NCOL // 2             # 94

f32 = mybir.dt.float32
bf16 = mybir.dt.bfloat16
bfnp = ml_dtypes.bfloat16


def build_kernel():
    nc = bacc.Bacc("TRN2", target_bir_lowering=False, debug=False,
                   num_devices=N_CORES)

    ins = []
    for k in range(SPC):
        ins.append({
            "zp": nc.dram_tensor(f"zp_{k}", [P, TPS * D], bf16,
                                 kind="ExternalInput"),
            "zt": nc.dram_tensor(f"zt_{k}", [P, TPS * P], bf16,
                                 kind="ExternalInput"),
            "rl": nc.dram_tensor(f"rl_{k}", [P, TPS * NLEV_LEAF * D], bf16,
                                 kind="ExternalInput"),
            "mk": nc.dram_tensor(f"mk_{k}", [P, TPS * NCOL], bf16,
                                 kind="ExternalInput"),
            "tb": nc.dram_tensor(f"tb_{k}", [P, TPS * NCOL], bf16,
                                 kind="ExternalInput"),
        })
    out = nc.dram_tensor("out", [P, SPC * TPS], f32, kind="ExternalOutput")

    with tile.TileContext(nc) as tc:
        with (
            tc.tile_pool(name="const", bufs=1) as cpool,
            tc.tile_pool(name="stream", bufs=2) as spool,
            tc.tile_pool(name="evp", bufs=2) as epool,
            tc.tile_pool(name="mmp", bufs=2) as wpool,
            tc.tile_pool(name="prodp", bufs=2) as ppool,
            tc.tile_pool(name="halfp", bufs=1) as hpool,
            tc.tile_pool(name="logp", bufs=2) as lpool,
            tc.tile_pool(name="psum", bufs=4, space="PSUM") as qpool,
        ):
            probs = cpool.tile([P, SPC * TPS], f32)

            for k in range(SPC):
                t_in = ins[k]
                zt = spool.tile([P, TPS, P], bf16, tag="zt")
                nc.sync.dma_start(out=zt[:], in_=t_in["zt"].ap().rearrange(
                    "p (t s) -> p t s", s=P))
                tb = spool.tile([P, TPS, NCOL], bf16, tag="tb")
                nc.sync.dma_start(out=tb[:], in_=t_in["tb"].ap().rearrange(
                    "p (t c) -> p t c", c=NCOL))
                mk = spool.tile([P, TPS, NCOL], bf16, tag="mk")
                nc.sync.dma_start(out=mk[:], in_=t_in["mk"].ap().rearrange(
                    "p (t c) -> p t c", c=NCOL))
                zp = spool.tile([P, TPS, D], bf16, tag="zp")
                nc.sync.dma_start(out=zp[:], in_=t_in["zp"].ap().rearrange(
                    "p (t d) -> p t d", d=D))
                rl = spool.tile([P, TPS, NLEV_LEAF * D], bf16, tag="rl")
                nc.sync.dma_start(out=rl[:], in_=t_in["rl"].ap().rearrange(
                    "p (t x) -> p t x", x=NLEV_LEAF * D))

                lg = lpool.tile([P, TPS, 21], f32, tag="lg")

                # --- leaf levels 15..20: per-sample dot via mult + tree ---
                for j in range(NCHUNK):
                    ts = j * TPCH
                    prod = ppool.tile([P, TPCH, NLEV_LEAF, D], bf16,
                                      tag="prod")
                    zc = zp[:, ts:ts + TPCH, :].unsqueeze(2)
                    nc.vector.tensor_tensor(
                        out=prod[:],
                        in0=zc.to_broadcast([P, TPCH, NLEV_LEAF, D]),
                        in1=rl[:, ts:ts + TPCH, :].rearrange(
                            "p t (l d) -> p t l d", d=D),
                        op=mybir.AluOpType.mult)
                    cur, width = prod, D
                    while width > 2:
                        width //= 2
                        nxt = hpool.tile([P, TPCH, NLEV_LEAF, width], bf16,
                                         tag=f"ph{width}", name=f"ph{width}")
                        nc.vector.tensor_tensor(
                            out=nxt[:], in0=cur[:, :, :, 0:width],
                            in1=cur[:, :, :, width:2 * width],
                            op=mybir.AluOpType.add)
                        cur = nxt
                    nc.vector.tensor_tensor(
                        out=lg[:, ts:ts + TPCH, NLEV_TAB:21],
                        in0=cur[:, :, :, 0], in1=cur[:, :, :, 1],
                        op=mybir.AluOpType.add)

                # --- tabled levels 0..14: matmul -> psum -> bf16 evac ---
                ev = epool.tile([P, TPS, NCOL], bf16, tag="ev")
                for g in range(NGRP):
                    pt = qpool.tile([P, TPG, NCOL], f32, tag="pt")
                    for i in range(TPG):
                        t = g * TPG + i
                        nc.tensor.matmul(pt[:, i, :], zt[:, t, :], tb[:, t, :],
                                         start=True, stop=True)
                    nc.scalar.copy(out=ev[:, g * TPG:(g + 1) * TPG, :],
                                   in_=pt[:])

                mm = wpool.tile([P, TPS, NCOL], bf16, tag="mm")
                nc.vector.tensor_tensor(out=mm[:], in0=ev[:], in1=mk[:],
                                        op=mybir.AluOpType.mult)
                # pair-split fold: slot pair members sit HNC apart (2x mode)
                p1 = wpool.tile([P, TPS, HNC], bf16, tag="p1")
                nc.vector.tensor_tensor(
                    out=p1[:], in0=mm[:, :, 0:HNC], in1=mm[:, :, HNC:NCOL],
                    op=mybir.AluOpType.add)
                # levels 0..8 fall out directly; ACT casts bf16 -> f32
                nc.scalar.copy(out=lg[:, :, 0:9], in_=p1[:, :, 0:9])
                for lev in range(9, NLEV_TAB):
                    off = int(SEG_OFF[lev]) // 2
                    w = SLOTS[lev] // 2
                    nc.vector.tensor_reduce(
                        out=lg[:, :, lev],
                        in_=p1[:, :, off:off + w],
                        axis=mybir.AxisListType.X, op=mybir.AluOpType.add)

                # --- sigmoid + product ---
                sg = lpool.tile([P, TPS, 32], f32, tag="sg")
                nc.vector.memset(sg[:, :, 21:32], 1.0)
                nc.scalar.activation(
                    out=sg[:, :, 0:21], in_=lg[:],
                    func=mybir.ActivationFunctionType.Sigmoid)
                cur, width = sg, 32
                while width > 2:
                    width //= 2
                    nxt = lpool.tile([P, TPS, width], f32, tag=f"s{width}",
                                     name=f"s{width}")
                    nc.vector.tensor_tensor(
                        out=nxt[:], in0=cur[:, :, 0:width],
                        in1=cur[:, :, width:2 * width],
                        op=mybir.AluOpType.mult)
                    cur = nxt
                nc.vector.tensor_tensor(
                    out=probs[:, k * TPS:(k + 1) * TPS], in0=cur[:, :, 0],
                    in1=cur[:, :, 1], op=mybir.AluOpType.mult)
                nc.sync.dma_start(
                    out=out.ap()[:, k * TPS:(k + 1) * TPS],
                    in_=probs[:, k * TPS:(k + 1) * TPS])

    nc.compile()
    return nc


_NC_CACHE = None


def _get_nc():
    global _NC_CACHE
    if _NC_CACHE is None:
        _NC_CACHE = build_kernel()
    return _NC_CACHE


def _ref_probs(collocation, W, idx):
    """Exact numpy fallback for slot-overflow samples (normally none)."""
    if len(idx) == 0:
        return np.zeros(0, dtype=np.float32)
    b = collocation[idx, 1].astype(np.int64) + OFFSET + 1
    z = W[collocation[idx, 0].astype(np.int64) + OFFSET]
    levels = np.arange(DEPTH + 1)
    path = (b[:, None] >> (DEPTH - levels)) - 1
    logits = np.einsum('bpd,bd->bp', W[path], z)
    return np.prod(1.0 / (1.0 + np.exp(-logits)), axis=-1).astype(np.float32)


def _prep(collocation, W):
    """Sort, build per-tile tables/masks, pre-gather rows. Returns
    (in_maps, order, fallback_original_indices)."""
    Wb = W.astype(bfnp)
    ctx = collocation[:, 1].astype(np.int64)
    z0 = collocation[:, 0].astype(np.int64)
    order = np.argsort(ctx, kind="stable")
    ctx_s = ctx[order]
    z0_s = z0[order]
    b = ctx_s + (1 << DEPTH)                     # 1-based leaf ids, sorted

    ntile = BATCH // P                           # 512
    tstart = np.arange(0, BATCH, P)

    nodes = np.zeros((ntile, NCOL), dtype=np.int64)
    M = np.zeros((BATCH, NCOL), dtype=bfnp)
    fallback = []
    rows = np.arange(BATCH)
    for lev in range(NLEV_TAB):
        a = b >> (DEPTH - lev)                   # 1-based ancestor ids
        ch = np.empty(BATCH, dtype=bool)
        ch[0] = True
        ch[1:] = a[1:] != a[:-1]
        ch[tstart] = True
        cs = np.cumsum(ch)
        slot = cs - np.repeat(cs[tstart], P)     # 0-based rank within tile
        ok = slot < SLOTS[lev]
        if not ok.all():
            fallback.append(rows[~ok])
        M[rows[ok], SEG_OFF[lev] + slot[ok]] = 1
        u = np.nonzero(ch)[0]
        uk = slot[u]
        uok = uk < SLOTS[lev]
        nodes[u[uok] >> 7, SEG_OFF[lev] + uk[uok]] = a[u[uok]] - 1

    # pair-split permutation: slot 2i -> off/2+i, slot 2i+1 -> HNC+off/2+i
    newpos = np.empty(NCOL, dtype=np.int64)
    for lev in range(NLEV_TAB):
        off = int(SEG_OFF[lev])
        w = SLOTS[lev]
        i = np.arange(w // 2)
        newpos[off + 2 * i] = off // 2 + i
        newpos[off + 2 * i + 1] = HNC + off // 2 + i
    nodes = nodes[:, np.argsort(newpos)]
    M = M[:, np.argsort(newpos)]

    Tb = Wb[nodes]                               # [ntile, NCOL, D]
    Z = Wb[z0_s + OFFSET]                        # [BATCH, D]
    R = np.empty((BATCH, NLEV_LEAF, D), dtype=bfnp)
    for i, lev in enumerate(range(NLEV_TAB, DEPTH + 1)):
        R[:, i, :] = Wb[(b >> (DEPTH - lev)) - 1]

    in_maps = []
    for c in range(N_CORES):
        m = {}
        for k in range(SPC):
            s = SPC * c + k
            sl = slice(SH * s, SH * (s + 1))
            z3 = Z[sl].reshape(TPS, P, D)
            m[f"zp_{k}"] = np.ascontiguousarray(
                z3.transpose(1, 0, 2)).reshape(P, TPS * D)
            m[f"zt_{k}"] = np.ascontiguousarray(
                z3.transpose(2, 0, 1)).reshape(P, TPS * P)
            m[f"rl_{k}"] = np.ascontiguousarray(
                R[sl].reshape(TPS, P, NLEV_LEAF * D).transpose(1, 0, 2)
            ).reshape(P, TPS * NLEV_LEAF * D)
            m[f"mk_{k}"] = np.ascontiguousarray(
                M[sl].reshape(TPS, P, NCOL).transpose(1, 0, 2)
            ).reshape(P, TPS * NCOL)
            m[f"tb_{k}"] = np.ascontiguousarray(
                Tb[TPS * s:TPS * (s + 1)].transpose(2, 0, 1)
            ).reshape(P, TPS * NCOL)
        in_maps.append(m)

    fb = (np.unique(np.concatenate(fallback)) if fallback
          else np.zeros(0, dtype=np.int64))
    return in_maps, order, fb


def _run(collocation: np.ndarray, W: np.ndarray, trace: bool = False,
         **spmd_kwargs):
    collocation = np.ascontiguousarray(collocation, dtype=np.int32)
    W = np.ascontiguousarray(W, dtype=np.float32)
    assert collocation.shape == (BATCH, 2)
    assert W.shape == ((1 << (DEPTH + 1)) - 1, D)

    nc = _get_nc()
    in_maps, order, fb = _prep(collocation, W)

    res = run_bass_kernel_spmd(
        nc, in_maps, core_ids=list(range(N_CORES)), trace=trace,
        **spmd_kwargs)

    out = np.empty(BATCH, dtype=np.float32)
    for c in range(N_CORES):
        oc = res.results[c]["out"]               # [128, 64]
        for k in range(SPC):
            s = SPC * c + k
            vals = oc[:, k * TPS:(k + 1) * TPS].T.reshape(SH)
            out[order[SH * s:SH * (s + 1)]] = vals
    if len(fb):
        oi = order[fb]
        out[oi] = _ref_probs(collocation, W, oi)
    return out, res


def kernel(collocation: np.ndarray, W: np.ndarray) -> np.ndarray:
    out, _ = _run(collocation, W, trace=False)
    return out


# revision 24
# speedup vs baseline: 1.3242x; 1.0321x over previous
"""v28: v26 + zp/rl transfers issued first (allocation order kept).

Base v14: host pre-gather + per-tile ancestor tables; no device-side gathers.

Host (data layout only, no FLOPs):
  Sort samples by context; 32 equal shards of 2048 (16 tiles of 128), core c
  runs shards 4c..4c+3.  For levels 0..14 the 128 sorted samples of a tile
  share few distinct ancestors, so each tile gets a node table (124 slot
  columns, per-level segment widths SLOTS) plus a per-sample one-hot mask.
  Leaf levels 15..20 rows and the z rows are pre-gathered per sample.

Device per shard (2048 samples):
  TensorE: psum[s, 124] = zt_tile^T @ table_tile   (dots vs all slot rows)
  ACT:     evacuate psum -> bf16
  DVE:     mm = ev * mask ; per-level segmented tensor_reduce -> logits 0..14
           prod = rows * z ; halving tree -> logits 15..20
  ACT:     sigmoid ; DVE: product tree -> probs
"""

import sys

for _p in ("/opt/trn_rl_repo", "/root/.axon_site/_ro/trn_rl_repo"):
    if _p not in sys.path:
        sys.path.append(_p)

import ml_dtypes
import numpy as np

import concourse.bacc as bacc
import concourse.mybir as mybir
import concourse.tile as tile
from concourse.bass_utils import run_bass_kernel_spmd

N_CORES = 8
BATCH = 65536
DEPTH = 20
OFFSET = (1 << DEPTH) - 1
D = 128
P = 128

SPC = 4                     # shards per core
NSHARD = N_CORES * SPC      # 32
SH = BATCH // NSHARD        # 2048 samples per shard
TPS = SH // P               # 16 tiles per shard
NLEV_TAB = 16               # levels 0..15 via per-tile tables
NLEV_LEAF = 5               # levels 16..20 pre-gathered rows
TPCH = 16                   # tiles per leaf chunk
NCHUNK = TPS // TPCH        # 4
NGRP = 8                    # psum groups per shard (2 tiles each)
TPG = TPS // NGRP

# slot widths per tabled level (0..8 get 2 each; deeper levels need more)
SLOTS = [2] * 9 + [4, 6, 8, 12, 24, 44, 72]
SEG_OFF = np.concatenate(([0], np.cumsum(SLOTS))).astype(np.int64)
NCOL = int(SEG_OFF[-1])     # 188
HNC = NCOL // 2             # 94

f32 = mybir.dt.float32
bf16 = mybir.dt.bfloat16
bfnp = ml_dtypes.bfloat16


def build_kernel():
    nc = bacc.Bacc("TRN2", target_bir_lowering=False, debug=False,
                   num_devices=N_CORES)

    ins = []
    for k in range(SPC):
        ins.append({
            "zp": nc.dram_tensor(f"zp_{k}", [P, TPS * D], bf16,
                                 kind="ExternalInput"),
            "zt": nc.dram_tensor(f"zt_{k}", [P, TPS * P], bf16,
                                 kind="ExternalInput"),
            "rl": nc.dram_tensor(f"rl_{k}", [P, TPS * NLEV_LEAF * D], bf16,
                                 kind="ExternalInput"),
            "mk": nc.dram_tensor(f"mk_{k}", [P, TPS * NCOL], bf16,
                                 kind="ExternalInput"),
            "tb": nc.dram_tensor(f"tb_{k}", [P, TPS * NCOL], bf16,
                                 kind="ExternalInput"),
        })
    out = nc.dram_tensor("out", [P, SPC * TPS], f32, kind="ExternalOutput")

    with tile.TileContext(nc) as tc:
        with (
            tc.tile_pool(name="const", bufs=1) as cpool,
            tc.tile_pool(name="stream", bufs=2) as spool,
            tc.tile_pool(name="evp", bufs=2) as epool,
            tc.tile_pool(name="mmp", bufs=2) as wpool,
            tc.tile_pool(name="prodp", bufs=2) as ppool,
            tc.tile_pool(name="halfp", bufs=1) as hpool,
            tc.tile_pool(name="logp", bufs=2) as lpool,
            tc.tile_pool(name="psum", bufs=4, space="PSUM") as qpool,
        ):
            probs = cpool.tile([P, SPC * TPS], f32)

            for k in range(SPC):
                t_in = ins[k]
                zt = spool.tile([P, TPS, P], bf16, tag="zt")
                tb = spool.tile([P, TPS, NCOL], bf16, tag="tb")
                mk = spool.tile([P, TPS, NCOL], bf16, tag="mk")
                zp = spool.tile([P, TPS, D], bf16, tag="zp")
                rl = spool.tile([P, TPS, NLEV_LEAF * D], bf16, tag="rl")
                nc.sync.dma_start(out=zp[:], in_=t_in["zp"].ap().rearrange(
                    "p (t d) -> p t d", d=D))
                nc.sync.dma_start(out=rl[:], in_=t_in["rl"].ap().rearrange(
                    "p (t x) -> p t x", x=NLEV_LEAF * D))
                nc.sync.dma_start(out=zt[:], in_=t_in["zt"].ap().rearrange(
                    "p (t s) -> p t s", s=P))
                nc.sync.dma_start(out=tb[:], in_=t_in["tb"].ap().rearrange(
                    "p (t c) -> p t c", c=NCOL))
                nc.sync.dma_start(out=mk[:], in_=t_in["mk"].ap().rearrange(
                    "p (t c) -> p t c", c=NCOL))

                lg = lpool.tile([P, TPS, 21], f32, tag="lg")

                # --- leaf levels 15..20: per-sample dot via mult + tree ---
                for j in range(NCHUNK):
                    ts = j * TPCH
                    prod = ppool.tile([P, TPCH, NLEV_LEAF, D], bf16,
                                      tag="prod")
                    zc = zp[:, ts:ts + TPCH, :].unsqueeze(2)
                    nc.vector.tensor_tensor(
                        out=prod[:],
                        in0=zc.to_broadcast([P, TPCH, NLEV_LEAF, D]),
                        in1=rl[:, ts:ts + TPCH, :].rearrange(
                            "p t (l d) -> p t l d", d=D),
                        op=mybir.AluOpType.mult)
                    cur, width = prod, D
                    while width > 2:
                        width //= 2
                        nxt = hpool.tile([P, TPCH, NLEV_LEAF, width], bf16,
                                         tag=f"ph{width}", name=f"ph{width}")
                        nc.vector.tensor_tensor(
                            out=nxt[:], in0=cur[:, :, :, 0:width],
                            in1=cur[:, :, :, width:2 * width],
                            op=mybir.AluOpType.add)
                        cur = nxt
                    nc.vector.tensor_tensor(
                        out=lg[:, ts:ts + TPCH, NLEV_TAB:21],
                        in0=cur[:, :, :, 0], in1=cur[:, :, :, 1],
                        op=mybir.AluOpType.add)

                # --- tabled levels 0..14: matmul -> psum -> bf16 evac ---
                ev = epool.tile([P, TPS, NCOL], bf16, tag="ev")
                for g in range(NGRP):
                    pt = qpool.tile([P, TPG, NCOL], f32, tag="pt")
                    for i in range(TPG):
                        t = g * TPG + i
                        nc.tensor.matmul(pt[:, i, :], zt[:, t, :], tb[:, t, :],
                                         start=True, stop=True)
                    nc.scalar.copy(out=ev[:, g * TPG:(g + 1) * TPG, :],
                                   in_=pt[:])

                mm = wpool.tile([P, TPS, NCOL], bf16, tag="mm")
                nc.vector.tensor_tensor(out=mm[:], in0=ev[:], in1=mk[:],
                                        op=mybir.AluOpType.mult)
                # pair-split fold: slot pair members sit HNC apart (2x mode)
                p1 = wpool.tile([P, TPS, HNC], bf16, tag="p1")
                nc.vector.tensor_tensor(
                    out=p1[:], in0=mm[:, :, 0:HNC], in1=mm[:, :, HNC:NCOL],
                    op=mybir.AluOpType.add)
                # levels 0..8 fall out directly; ACT casts bf16 -> f32
                nc.scalar.copy(out=lg[:, :, 0:9], in_=p1[:, :, 0:9])
                for lev in range(9, NLEV_TAB):
                    off = int(SEG_OFF[lev]) // 2
                    w = SLOTS[lev] // 2
                    nc.vector.tensor_reduce(
                        out=lg[:, :, lev],
                        in_=p1[:, :, off:off + w],
                        axis=mybir.AxisListType.X, op=mybir.AluOpType.add)

                # --- sigmoid + product ---
                sg = lpool.tile([P, TPS, 32], f32, tag="sg")
                nc.vector.memset(sg[:, :, 21:32], 1.0)
                nc.scalar.activation(
                    out=sg[:, :, 0:21], in_=lg[:],
                    func=mybir.ActivationFunctionType.Sigmoid)
                cur, width = sg, 32
                while width > 2:
                    width //= 2
                    nxt = lpool.tile([P, TPS, width], f32, tag=f"s{width}",
                                     name=f"s{width}")
                    nc.vector.tensor_tensor(
                        out=nxt[:], in0=cur[:, :, 0:width],
                        in1=cur[:, :, width:2 * width],
                        op=mybir.AluOpType.mult)
                    cur = nxt
                nc.vector.tensor_tensor(
                    out=probs[:, k * TPS:(k + 1) * TPS], in0=cur[:, :, 0],
                    in1=cur[:, :, 1], op=mybir.AluOpType.mult)
                nc.sync.dma_start(
                    out=out.ap()[:, k * TPS:(k + 1) * TPS],
                    in_=probs[:, k * TPS:(k + 1) * TPS])

    nc.compile()
    return nc


_NC_CACHE = None


def _get_nc():
    global _NC_CACHE
    if _NC_CACHE is None:
        _NC_CACHE = build_kernel()
    return _NC_CACHE


def _ref_probs(collocation, W, idx):
    """Exact numpy fallback for slot-overflow samples (normally none)."""
    if len(idx) == 0:
        return np.zeros(0, dtype=np.float32)
    b = collocation[idx, 1].astype(np.int64) + OFFSET + 1
    z = W[collocation[idx, 0].astype(np.int64) + OFFSET]
    levels = np.arange(DEPTH + 1)
    path = (b[:, None] >> (DEPTH - levels)) - 1
    logits = np.einsum('bpd,bd->bp', W[path], z)
    return np.prod(1.0 / (1.0 + np.exp(-logits)), axis=-1).astype(np.float32)


def _prep(collocation, W):
    """Sort, build per-tile tables/masks, pre-gather rows. Returns
    (in_maps, order, fallback_original_indices)."""
    Wb = W.astype(bfnp)
    ctx = collocation[:, 1].astype(np.int64)
    z0 = collocation[:, 0].astype(np.int64)
    order = np.argsort(ctx, kind="stable")
    ctx_s = ctx[order]
    z0_s = z0[order]
    b = ctx_s + (1 << DEPTH)                     # 1-based leaf ids, sorted

    ntile = BATCH // P                           # 512
    tstart = np.arange(0, BATCH, P)

    nodes = np.zeros((ntile, NCOL), dtype=np.int64)
    M = np.zeros((BATCH, NCOL), dtype=bfnp)
    fallback = []
    rows = np.arange(BATCH)
    for lev in range(NLEV_TAB):
        a = b >> (DEPTH - lev)                   # 1-based ancestor ids
        ch = np.empty(BATCH, dtype=bool)
        ch[0] = True
        ch[1:] = a[1:] != a[:-1]
        ch[tstart] = True
        cs = np.cumsum(ch)
        slot = cs - np.repeat(cs[tstart], P)     # 0-based rank within tile
        ok = slot < SLOTS[lev]
        if not ok.all():
            fallback.append(rows[~ok])
        M[rows[ok], SEG_OFF[lev] + slot[ok]] = 1
        u = np.nonzero(ch)[0]
        uk = slot[u]
        uok = uk < SLOTS[lev]
        nodes[u[uok] >> 7, SEG_OFF[lev] + uk[uok]] = a[u[uok]] - 1

    # pair-split permutation: slot 2i -> off/2+i, slot 2i+1 -> HNC+off/2+i
    newpos = np.empty(NCOL, dtype=np.int64)
    for lev in range(NLEV_TAB):
        off = int(SEG_OFF[lev])
        w = SLOTS[lev]
        i = np.arange(w // 2)
        newpos[off + 2 * i] = off // 2 + i
        newpos[off + 2 * i + 1] = HNC + off // 2 + i
    nodes = nodes[:, np.argsort(newpos)]
    M = M[:, np.argsort(newpos)]

    Tb = Wb[nodes]                               # [ntile, NCOL, D]
    Z = Wb[z0_s + OFFSET]                        # [BATCH, D]
    R = np.empty((BATCH, NLEV_LEAF, D), dtype=bfnp)
    for i, lev in enumerate(range(NLEV_TAB, DEPTH + 1)):
        R[:, i, :] = Wb[(b >> (DEPTH - lev)) - 1]

    in_maps = []
    for c in range(N_CORES):
        m = {}
        for k in range(SPC):
            s = SPC * c + k
            sl = slice(SH * s, SH * (s + 1))
            z3 = Z[sl].reshape(TPS, P, D)
            m[f"zp_{k}"] = np.ascontiguousarray(
                z3.transpose(1, 0, 2)).reshape(P, TPS * D)
            m[f"zt_{k}"] = np.ascontiguousarray(
                z3.transpose(2, 0, 1)).reshape(P, TPS * P)
            m[f"rl_{k}"] = np.ascontiguousarray(
                R[sl].reshape(TPS, P, NLEV_LEAF * D).transpose(1, 0, 2)
            ).reshape(P, TPS * NLEV_LEAF * D)
            m[f"mk_{k}"] = np.ascontiguousarray(
                M[sl].reshape(TPS, P, NCOL).transpose(1, 0, 2)
            ).reshape(P, TPS * NCOL)
            m[f"tb_{k}"] = np.ascontiguousarray(
                Tb[TPS * s:TPS * (s + 1)].transpose(2, 0, 1)
            ).reshape(P, TPS * NCOL)
        in_maps.append(m)

    fb = (np.unique(np.concatenate(fallback)) if fallback
          else np.zeros(0, dtype=np.int64))
    return in_maps, order, fb


def _run(collocation: np.ndarray, W: np.ndarray, trace: bool = False,
         **spmd_kwargs):
    collocation = np.ascontiguousarray(collocation, dtype=np.int32)
    W = np.ascontiguousarray(W, dtype=np.float32)
    assert collocation.shape == (BATCH, 2)
    assert W.shape == ((1 << (DEPTH + 1)) - 1, D)

    nc = _get_nc()
    in_maps, order, fb = _prep(collocation, W)

    res = run_bass_kernel_spmd(
        nc, in_maps, core_ids=list(range(N_CORES)), trace=trace,
        **spmd_kwargs)

    out = np.empty(BATCH, dtype=np.float32)
    for c in range(N_CORES):
        oc = res.results[c]["out"]               # [128, 64]
        for k in range(SPC):
            s = SPC * c + k
            vals = oc[:, k * TPS:(k + 1) * TPS].T.reshape(SH)
            out[order[SH * s:SH * (s + 1)]] = vals
    if len(fb):
        oi = order[fb]
        out[oi] = _ref_probs(collocation, W, oi)
    return out, res


def kernel(collocation: np.ndarray, W: np.ndarray) -> np.ndarray:
    out, _ = _run(collocation, W, trace=False)
    return out
